# revision 13
# baseline (speedup 1.0000x reference)
"""Multi-head attention (B=2, S=2048, H=16, D=128, fp32, non-causal) on 8
Trainium2 NeuronCores.

Strategy: the 32 (batch, head) pairs are independent -> head-parallel
(Ulysses-style) sharding, 4 pairs per core, no on-device collectives.
The host pre-transposes Q and K to [d, s] layout per pair (so the
contraction dim d lands on SBUF partitions with no on-chip transposes),
and the kernel produces out^T [d, s] which the host transposes back.

v2 engine balance (the v1 kernel was ACT-bound at 93% busy):
- exp of scores is split between the ACT engine (table exp, fp16 out) and
  the DVE via a Schraudolph bit-trick: one tensor_scalar computing
  round(score * SCALE*1024/ln2 + 15316) into an int16 tile whose bits ARE
  the fp16 exp approximation (max rel err ~3%, which perturbs the softmax
  output by <1e-3 of its max; conversion rounding verified on HW).
- softmax reciprocal moved off ACT entirely: 1/sums via a fast-inverse
  int32 bit-trick seed + one Newton step, all on the otherwise-idle GPSIMD
  engine (stock tensor ops; ~1.2e-3 rel err).
- the final normalize multiply is emitted one q-block late so the
  in-order DVE/GPSIMD queues never stall the steady-state exp pipeline.
- ACT runs a pure exp stream (no Ln/Exp reciprocal chain, no stalls).
"""

import math

import numpy as np

B, S, H, D = 2, 2048, 16, 128
N_CORES = 8
PAIRS_PER_CORE = (B * H) // N_CORES  # 4
P = 128
QBLK = 512  # q columns per q-block (one PSUM bank of fp32)
N_QB = S // QBLK  # 4
N_SK = S // P  # 16 sk tiles per pair
SK_PER_GROUP = 2  # sk tiles per scores/exp group ([128, 1024] psum tiles)
N_GROUPS = N_SK // SK_PER_GROUP  # 8
GW = SK_PER_GROUP * QBLK  # group width: 1024
SCALE = 1.0 / math.sqrt(D)

# group whose exp runs on DVE instead of ACT (None = all ACT). Its PV
# matmuls and softmax-sum add are deferred to the end of the q-block, so the
# DVE latency hides behind the other 7 groups' compute. Group 3 is chosen so
# the DVE op sits mid-stream, clear of the q-block-boundary critical path.
OFF_GROUP = 3

# Schraudolph fp16 exp: bits16 = round(x * EXP_A + EXP_B); bitcast -> fp16
EXP_A = 1024.0 / math.log(2.0)
EXP_B = 15360.0 - 44.0
# fast inverse seed: y0_bits = RECIP_MAGIC - bits(x)
RECIP_MAGIC = 0x7EF311C3

_COMPILED = None


def _patch_tile_drain():
    """Workaround for walrus 'Too many sync wait commands' on the TileContext
    tail Drain: redistribute all but one of the drain's sem waits onto
    single-wait NoOps on the sync engine (program order places them after the
    drain and before the all-engine barrier, which preserves semantics)."""
    import concourse.mybir as mybir
    import concourse.tile as tile
    from concourse.vector_clock import ScopedClock

    if getattr(tile.TileContext, "_ant_drain_patched", False):
        return

    def _drain_and_barrier(self, tick_clock, wait_clock):
        drain_inst = self.nc.sync.drain()
        wait_clock.add_sem_waits(
            drain_inst.ins, ScopedClock({None: tick_clock.global_clock})
        )
        si = drain_inst.ins.sync_info
        if si is not None and si.on_wait and len(si.on_wait) > 1:
            waits = list(si.on_wait)
            si.on_wait = waits[:1]
            # distribute the remaining waits round-robin across engines so
            # they are honored in parallel; the all-engine barrier below
            # collects them all before the semaphore reset
            engines = [
                self.nc.sync, self.nc.vector, self.nc.scalar,
                self.nc.tensor, self.nc.gpsimd,
            ]
            for i, w in enumerate(waits[1:]):
                nop = engines[i % len(engines)].nop(nofuse=True)
                nop.ins.sync_info = mybir.SyncInfo(on_wait=[w], on_update=[])

        self.nc.all_engine_barrier()
        assert self.sems is not None
        popped = self.nc._tile_sem_poison_stack.pop()
        assert popped is self._sem_poison
        self.nc.clear_and_free_semaphores(list(self.sems.allocated().values()))
        self.nc.all_engine_barrier()

    tile.TileContext._drain_and_barrier = _drain_and_barrier
    tile.TileContext._ant_drain_patched = True


def _split_excess_waits(nc):
    """This container's walrus rejects instructions carrying more than a
    struct-dependent number of semaphore waits (setupSyncWait: 'Too many
    sync wait commands'): 1 for Matmult/Ldweights (S3_LW struct), 2 for
    everything else. Hoist the excess onto NoOps inserted just before the
    instruction on the same engine — same-engine program order guarantees
    they are honored before the instruction issues."""
    import concourse.mybir as mybir

    seq = 0
    for f in nc.m.functions:
        for b in f.blocks:
            insts = list(b.instructions)
            out = []
            changed = False
            for inst in insts:
                max_waits = 1
                si = inst.sync_info
                if si is not None and si.on_wait and len(si.on_wait) > max_waits:
                    waits = list(si.on_wait)
                    si.on_wait = waits[:max_waits]
                    # NoOps (CTRL struct) only take 1 wait each
                    for w in waits[max_waits:]:
                        nop = mybir.InstNoOp(name=f"ant-waitsplit-{seq}")
                        seq += 1
                        nop.engine = inst.engine
                        nop.sync_info = mybir.SyncInfo(
                            on_wait=[w], on_update=[]
                        )
                        out.append(nop)
                    changed = True
                out.append(inst)
            if changed:
                b.instructions = out
    return nc


def _build():
    import concourse.bass as bass
    import concourse.mybir as mybir
    import concourse.tile as tile

    _patch_tile_drain()

    f32 = mybir.dt.float32
    f32r = mybir.dt.float32r
    f16 = mybir.dt.float16
    i16 = mybir.dt.int16
    i32 = mybir.dt.int32
    mult = mybir.AluOpType.mult
    add = mybir.AluOpType.add
    subtract = mybir.AluOpType.subtract
    nc = bass.Bass()

    # Q/K arrive pre-rounded to the fp32r grid (RNE at 11 mantissa bits,
    # verified bit-exact against the on-chip DVE cast) so they DMA straight
    # into fp32r tiles; V arrives pre-cast to fp16.
    qT = nc.dram_tensor("qT", [PAIRS_PER_CORE, P, S], f32r, kind="ExternalInput")
    kT = nc.dram_tensor("kT", [PAIRS_PER_CORE, P, S], f32r, kind="ExternalInput")
    v = nc.dram_tensor("v", [PAIRS_PER_CORE, S, D], f16, kind="ExternalInput")
    outT = nc.dram_tensor("outT", [PAIRS_PER_CORE, P, S], f32, kind="ExternalOutput")

    with tile.TileContext(nc) as tc:
        with (
            tc.tile_pool(name="const", bufs=1) as const_pool,
            tc.tile_pool(name="inp", bufs=2) as inp_pool,
            tc.tile_pool(name="exp", bufs=8) as exp_pool,
            tc.tile_pool(name="acc", bufs=2) as acc_pool,
            tc.tile_pool(name="norm", bufs=2) as norm_pool,
            tc.tile_pool(name="outsb", bufs=3) as out_pool,
            tc.tile_pool(name="sc_ps", bufs=3, space="PSUM") as sc_psum,
            tc.tile_pool(name="o_ps", bufs=2, space="PSUM") as o_psum,
        ):
            ones_ld = const_pool.tile([P, P], f32)
            nc.vector.memset(ones_ld[:], 1.0)
            ones = const_pool.tile([P, P], f16)
            nc.vector.tensor_copy(ones[:], ones_ld[:])

            def emit_loads(pair):
                # chunked so the first scores matmuls start sooner: the
                # first q-block needs qT[:, :512] and kT tiles in order
                qT_sb = inp_pool.tile([P, S], f32r, tag="qT")
                kT_sb = inp_pool.tile([P, S], f32r, tag="kT")
                v_sb = inp_pool.tile([P, N_SK, D], f16, tag="v")
                nQ = 4
                for h in range(nQ):
                    sl = slice(h * (S // nQ), (h + 1) * (S // nQ))
                    nc.sync.dma_start(kT_sb[:, sl], kT[pair][:, sl])
                    if h == 0:
                        nc.sync.dma_start(qT_sb[:, sl], qT[pair][:, sl])
                rest = slice(S // nQ, S)
                nc.sync.dma_start(qT_sb[:, rest], qT[pair][:, rest])
                nc.sync.dma_start(
                    v_sb[:], v[pair].rearrange("(t p) d -> p t d", p=P)
                )
                return qT_sb, kT_sb, v_sb

            # pending normalize from the previous q-block:
            # (out_ps tile, -1/sums tile, pair idx, q slice)
            # deferred-by-one-q-block pipelines: nothing dependency-heavy is
            # ever emitted at the tail of a q-block, so no engine's in-order
            # queue blocks the next q-block's scores/exp stream.
            pending_sums = []  # (acc, out_ps, pair, q_sl) awaiting sums chain
            pending_mul = []   # (out_ps, y1n, pair, q_sl) awaiting normalize

            def flush_mul():
                while pending_mul:
                    out_ps, y1n, ppair, psl = pending_mul.pop(0)
                    o_sb = out_pool.tile([P, QBLK], f32, tag="osb")
                    nc.vector.tensor_mul(o_sb[:], out_ps[:], y1n[:])
                    nc.sync.dma_start(outT[ppair][:, psl], o_sb[:])

            def flush_sums():
                while pending_sums:
                    acc, out_ps, ppair, psl = pending_sums.pop(0)
                    # partition-reduce both acc halves into one PSUM bank
                    # (2-matmul accumulation; no separate fold op needed);
                    # the sums tile squats in the sc ring, keeping PSUM at
                    # exactly 8 banks with triple-buffered scores
                    sums_ps = sc_psum.tile([P, GW], f32, tag="sc")
                    nc.tensor.matmul(
                        sums_ps[:, :QBLK], ones[:], acc[:, :QBLK],
                        start=True, stop=False,
                    )
                    nc.tensor.matmul(
                        sums_ps[:, :QBLK], ones[:], acc[:, QBLK:],
                        start=False, stop=True,
                    )
                    sums_sb = norm_pool.tile([P, QBLK], f32, tag="sums_sb")
                    nc.vector.tensor_copy(sums_sb[:], sums_ps[:, :QBLK])
                    # fast inverse: bit-trick seed + 1 Newton step on GPSIMD
                    y0i = norm_pool.tile([P, QBLK], i32, tag="y0")
                    nc.gpsimd.tensor_scalar(
                        y0i[:], sums_sb[:].bitcast(i32), -1, RECIP_MAGIC,
                        mult, add,
                    )
                    y0f = y0i[:].bitcast(f32)
                    t2 = norm_pool.tile([P, QBLK], f32, tag="t2")
                    nc.gpsimd.tensor_tensor(t2[:], sums_sb[:], y0f, mult)
                    u2 = norm_pool.tile([P, QBLK], f32, tag="u2")
                    nc.gpsimd.tensor_scalar(
                        u2[:], t2[:], -1.0, 2.0, mult, add
                    )
                    y1n = norm_pool.tile([P, QBLK], f32, tag="y1n")
                    nc.gpsimd.tensor_tensor(y1n[:], u2[:], y0f, mult)
                    pending_mul.append((out_ps, y1n, ppair, psl))

            # software prefetch: emit the next pair's load DMAs before the
            # current pair's compute so transfers fully overlap it
            cur_tiles = emit_loads(0)
            for pair in range(PAIRS_PER_CORE):
                qT_sb, kT_sb, v_sb = cur_tiles
                if pair + 1 < PAIRS_PER_CORE:
                    cur_tiles = emit_loads(pair + 1)

                for qb in range(N_QB):
                    q_sl = slice(qb * QBLK, (qb + 1) * QBLK)
                    # normalize+store for q-block i-2 (its 1/sums just
                    # finished on GPSIMD during q-block i-1)
                    flush_mul()
                    out_ps = o_psum.tile([P, QBLK], f32, tag="ops")
                    acc = acc_pool.tile([P, GW], f16, tag="acc")

                    def emit_pv(g, ef, start=False, stop=False):
                        for j in range(SK_PER_GROUP):
                            sk = g * SK_PER_GROUP + j
                            nc.tensor.matmul(
                                out_ps[:],
                                v_sb[:, sk, :],
                                ef[:, j * QBLK : (j + 1) * QBLK],
                                start=(start and j == 0),
                                stop=(stop and j == SK_PER_GROUP - 1),
                            )

                    # software-pipelined: PV matmuls for group g-1 are
                    # emitted after the scores matmuls of group g, so the PE
                    # never stalls on the exp of the current group.
                    # OFF_GROUP's exp runs on DVE (Schraudolph); its PV pair
                    # and sum-add are deferred to the end of the q-block so
                    # the DVE latency hides behind the other groups.
                    e_tiles = [None] * N_GROUPS
                    first_pv = next(
                        g for g in range(N_GROUPS) if g != OFF_GROUP
                    )
                    for g in range(N_GROUPS):
                        sc = sc_psum.tile([P, GW], f32, tag="sc")
                        for j in range(SK_PER_GROUP):
                            sk = g * SK_PER_GROUP + j
                            nc.tensor.matmul(
                                sc[:, j * QBLK : (j + 1) * QBLK],
                                kT_sb[:, sk * P : (sk + 1) * P],
                                qT_sb[:, q_sl],
                                start=True,
                                stop=True,
                            )
                        ei = exp_pool.tile([P, GW], i16, tag="e")
                        ef = ei[:].bitcast(f16)
                        if g == OFF_GROUP:
                            # Schraudolph: fp16 exp bits via one tensor_scalar
                            nc.vector.tensor_scalar(
                                ei[:], sc[:], EXP_A * SCALE, EXP_B,
                                mult, add,
                            )
                        else:
                            nc.scalar.activation(
                                ef, sc[:],
                                mybir.ActivationFunctionType.Exp,
                                scale=SCALE,
                            )
                        e_tiles[g] = ef
                        if g == 2:
                            # previous q-block's sums chain: emitted a few
                            # groups into this q-block so its acc dependency
                            # (complete ~2us past the boundary) never stalls
                            # the PE stream
                            flush_sums()
                        # softmax-sum adds strictly in group order (the
                        # OFF_GROUP add follows its DVE exp immediately), so
                        # acc completes one add after the last ACT exp
                        if g == 1:
                            nc.vector.tensor_add(
                                acc[:], e_tiles[0], e_tiles[1]
                            )
                        elif g > 1:
                            nc.vector.tensor_add(acc[:], acc[:], ef)
                        if g >= 1 and (g - 1) != OFF_GROUP and g != first_pv:
                            emit_pv(
                                g - 1, e_tiles[g - 1],
                                start=(g - 1 == first_pv),
                            )
                    emit_pv(N_GROUPS - 1, e_tiles[N_GROUPS - 1])
                    emit_pv(OFF_GROUP, e_tiles[OFF_GROUP], stop=True)
                    pending_sums.append((acc, out_ps, pair, q_sl))

            flush_sums()
            flush_mul()

    _split_excess_waits(nc)
    return nc


def _get_compiled():
    global _COMPILED
    if _COMPILED is None:
        _COMPILED = _build()
    return _COMPILED


def _round_f32r(x):
    """Round fp32 to the fp32r grid: round-to-nearest-even at 11 mantissa
    bits (verified bit-exact against the on-chip DVE fp32->fp32r cast)."""
    b = np.ascontiguousarray(x).view(np.uint32).astype(np.uint64)
    drop = np.uint64(12)
    half = np.uint64(1 << 11)
    lsb = (b >> drop) & np.uint64(1)
    r = (b + half - np.uint64(1) + lsb) & np.uint64(0xFFFFF000)
    return r.astype(np.uint32).view(np.float32).reshape(x.shape)


def _shard_inputs(query, key, value):
    """Full [B,S,H,D] inputs -> per-core input maps (host-side Ulysses)."""
    # [B,S,H,D] -> [B,H,D,S] -> [BH, D, S] for q/k; [B,H,S,D] -> [BH, S, D] for v
    qT_all = np.ascontiguousarray(np.transpose(query, (0, 2, 3, 1))).reshape(
        B * H, D, S
    )
    kT_all = np.ascontiguousarray(np.transpose(key, (0, 2, 3, 1))).reshape(
        B * H, D, S
    )
    v_all = np.ascontiguousarray(np.transpose(value, (0, 2, 1, 3))).reshape(
        B * H, S, D
    )
    in_maps = []
    for c in range(N_CORES):
        sl = slice(c * PAIRS_PER_CORE, (c + 1) * PAIRS_PER_CORE)
        in_maps.append(
            {
                "qT": _round_f32r(qT_all[sl]),
                "kT": _round_f32r(kT_all[sl]),
                "v": np.ascontiguousarray(v_all[sl]).astype(np.float16),
            }
        )
    return in_maps


def _gather_output(results):
    outT_all = np.concatenate([r["outT"] for r in results], axis=0)  # [BH, D, S]
    out = outT_all.reshape(B, H, D, S).transpose(0, 3, 1, 2)  # [B, S, H, D]
    return np.ascontiguousarray(out)


def kernel(query, key, value, _run_kwargs=None):
    from concourse.bass_utils import run_bass_kernel_spmd

    nc = _get_compiled()
    in_maps = _shard_inputs(
        np.asarray(query, dtype=np.float32),
        np.asarray(key, dtype=np.float32),
        np.asarray(value, dtype=np.float32),
    )
    kwargs = _run_kwargs or {}
    res = run_bass_kernel_spmd(nc, in_maps, core_ids=list(range(N_CORES)), **kwargs)
    out = _gather_output(res.results)
    if _run_kwargs is not None:
        kernel.last_result = res
    return out


# revision 14
# speedup vs baseline: 1.1909x; 1.1909x over previous
"""Multi-head attention (B=2, S=2048, H=16, D=128, fp32, non-causal) on 8
Trainium2 NeuronCores.

Strategy: the 32 (batch, head) pairs are independent -> head-parallel
(Ulysses-style) sharding, 4 pairs per core, no on-device collectives.
The host pre-transposes Q and K to [d, s] layout per pair (so the
contraction dim d lands on SBUF partitions with no on-chip transposes),
and the kernel produces out^T [d, s] which the host transposes back.

v2 engine balance (the v1 kernel was ACT-bound at 93% busy):
- exp of scores is split between the ACT engine (table exp, fp16 out) and
  the DVE via a Schraudolph bit-trick: one tensor_scalar computing
  round(score * SCALE*1024/ln2 + 15316) into an int16 tile whose bits ARE
  the fp16 exp approximation (max rel err ~3%, which perturbs the softmax
  output by <1e-3 of its max; conversion rounding verified on HW).
- softmax reciprocal moved off ACT entirely: 1/sums via a fast-inverse
  int32 bit-trick seed + one Newton step, all on the otherwise-idle GPSIMD
  engine (stock tensor ops; ~1.2e-3 rel err).
- the final normalize multiply is emitted one q-block late so the
  in-order DVE/GPSIMD queues never stall the steady-state exp pipeline.
- ACT runs a pure exp stream (no Ln/Exp reciprocal chain, no stalls).
"""

import math

import numpy as np

B, S, H, D = 2, 2048, 16, 128
N_CORES = 8
PAIRS_PER_CORE = (B * H) // N_CORES  # 4
P = 128
QBLK = 512  # q columns per q-block (one PSUM bank of fp32)
N_QB = S // QBLK  # 4
N_SK = S // P  # 16 sk tiles per pair
SK_PER_GROUP = 2  # sk tiles per scores/exp group ([128, 1024] psum tiles)
N_GROUPS = N_SK // SK_PER_GROUP  # 8
GW = SK_PER_GROUP * QBLK  # group width: 1024
SCALE = 1.0 / math.sqrt(D)

# group whose exp runs on DVE instead of ACT (None = all ACT). Its PV
# matmuls and softmax-sum add are deferred to the end of the q-block, so the
# DVE latency hides behind the other 7 groups' compute. Group 3 is chosen so
# the DVE op sits mid-stream, clear of the q-block-boundary critical path.
OFF_GROUP = 3

# Schraudolph fp16 exp: bits16 = round(x * EXP_A + EXP_B); bitcast -> fp16
EXP_A = 1024.0 / math.log(2.0)
EXP_B = 15360.0 - 44.0
# fast inverse seed: y0_bits = RECIP_MAGIC - bits(x)
RECIP_MAGIC = 0x7EF311C3

_COMPILED = None


def _patch_tile_drain():
    """Workaround for walrus 'Too many sync wait commands' on the TileContext
    tail Drain: redistribute all but one of the drain's sem waits onto
    single-wait NoOps on the sync engine (program order places them after the
    drain and before the all-engine barrier, which preserves semantics)."""
    import concourse.mybir as mybir
    import concourse.tile as tile
    from concourse.vector_clock import ScopedClock

    if getattr(tile.TileContext, "_ant_drain_patched", False):
        return

    def _drain_and_barrier(self, tick_clock, wait_clock):
        drain_inst = self.nc.sync.drain()
        wait_clock.add_sem_waits(
            drain_inst.ins, ScopedClock({None: tick_clock.global_clock})
        )
        si = drain_inst.ins.sync_info
        if si is not None and si.on_wait and len(si.on_wait) > 1:
            waits = list(si.on_wait)
            si.on_wait = waits[:1]
            # distribute the remaining waits round-robin across engines so
            # they are honored in parallel; the all-engine barrier below
            # collects them all before the semaphore reset
            engines = [
                self.nc.sync, self.nc.vector, self.nc.scalar,
                self.nc.tensor, self.nc.gpsimd,
            ]
            for i, w in enumerate(waits[1:]):
                nop = engines[i % len(engines)].nop(nofuse=True)
                nop.ins.sync_info = mybir.SyncInfo(on_wait=[w], on_update=[])

        self.nc.all_engine_barrier()
        assert self.sems is not None
        popped = self.nc._tile_sem_poison_stack.pop()
        assert popped is self._sem_poison
        self.nc.clear_and_free_semaphores(list(self.sems.allocated().values()))
        self.nc.all_engine_barrier()

    tile.TileContext._drain_and_barrier = _drain_and_barrier
    tile.TileContext._ant_drain_patched = True


def _split_excess_waits(nc):
    """This container's walrus rejects instructions carrying more than a
    struct-dependent number of semaphore waits (setupSyncWait: 'Too many
    sync wait commands'): 1 for Matmult/Ldweights (S3_LW struct), 2 for
    everything else. Hoist the excess onto NoOps inserted just before the
    instruction on the same engine — same-engine program order guarantees
    they are honored before the instruction issues."""
    import concourse.mybir as mybir

    seq = 0
    for f in nc.m.functions:
        for b in f.blocks:
            insts = list(b.instructions)
            out = []
            changed = False
            for inst in insts:
                max_waits = 1
                si = inst.sync_info
                if si is not None and si.on_wait and len(si.on_wait) > max_waits:
                    waits = list(si.on_wait)
                    si.on_wait = waits[:max_waits]
                    # NoOps (CTRL struct) only take 1 wait each
                    for w in waits[max_waits:]:
                        nop = mybir.InstNoOp(name=f"ant-waitsplit-{seq}")
                        seq += 1
                        nop.engine = inst.engine
                        nop.sync_info = mybir.SyncInfo(
                            on_wait=[w], on_update=[]
                        )
                        out.append(nop)
                    changed = True
                out.append(inst)
            if changed:
                b.instructions = out
    return nc


def _build():
    import concourse.bass as bass
    import concourse.mybir as mybir
    import concourse.tile as tile

    _patch_tile_drain()

    f32 = mybir.dt.float32
    f32r = mybir.dt.float32r
    f16 = mybir.dt.float16
    i16 = mybir.dt.int16
    i32 = mybir.dt.int32
    mult = mybir.AluOpType.mult
    add = mybir.AluOpType.add
    subtract = mybir.AluOpType.subtract
    nc = bass.Bass()

    # Q/K arrive pre-rounded to the fp32r grid (RNE at 11 mantissa bits,
    # verified bit-exact against the on-chip DVE cast) so they DMA straight
    # into fp32r tiles; V arrives pre-cast to fp16.
    qT = nc.dram_tensor("qT", [PAIRS_PER_CORE, P, S], f32r, kind="ExternalInput")
    kT = nc.dram_tensor("kT", [PAIRS_PER_CORE, P, S], f32r, kind="ExternalInput")
    v = nc.dram_tensor("v", [PAIRS_PER_CORE, S, D], f16, kind="ExternalInput")
    outT = nc.dram_tensor("outT", [PAIRS_PER_CORE, P, S], f32, kind="ExternalOutput")

    with tile.TileContext(nc) as tc:
        with (
            tc.tile_pool(name="const", bufs=1) as const_pool,
            tc.tile_pool(name="inp", bufs=2) as inp_pool,
            tc.tile_pool(name="exp", bufs=8) as exp_pool,
            tc.tile_pool(name="acc", bufs=2) as acc_pool,
            tc.tile_pool(name="norm", bufs=2) as norm_pool,
            tc.tile_pool(name="outsb", bufs=3) as out_pool,
            tc.tile_pool(name="sc_ps", bufs=3, space="PSUM") as sc_psum,
            tc.tile_pool(name="o_ps", bufs=2, space="PSUM") as o_psum,
        ):
            ones_ld = const_pool.tile([P, P], f32)
            nc.vector.memset(ones_ld[:], 1.0)
            ones = const_pool.tile([P, P], f16)
            nc.vector.tensor_copy(ones[:], ones_ld[:])

            def emit_loads(pair):
                # chunked so the first scores matmuls start sooner: the
                # first q-block needs qT[:, :512] and kT tiles in order
                qT_sb = inp_pool.tile([P, S], f32r, tag="qT")
                kT_sb = inp_pool.tile([P, S], f32r, tag="kT")
                v_sb = inp_pool.tile([P, N_SK, D], f16, tag="v")
                nQ = 4
                for h in range(nQ):
                    sl = slice(h * (S // nQ), (h + 1) * (S // nQ))
                    nc.sync.dma_start(kT_sb[:, sl], kT[pair][:, sl])
                    if h == 0:
                        nc.sync.dma_start(qT_sb[:, sl], qT[pair][:, sl])
                rest = slice(S // nQ, S)
                nc.sync.dma_start(qT_sb[:, rest], qT[pair][:, rest])
                nc.sync.dma_start(
                    v_sb[:], v[pair].rearrange("(t p) d -> p t d", p=P)
                )
                return qT_sb, kT_sb, v_sb

            # pending normalize from the previous q-block:
            # (out_ps tile, -1/sums tile, pair idx, q slice)
            # deferred-by-one-q-block pipelines: nothing dependency-heavy is
            # ever emitted at the tail of a q-block, so no engine's in-order
            # queue blocks the next q-block's scores/exp stream.
            pending_sums = []  # (acc, out_ps, pair, q_sl) awaiting sums chain
            pending_mul = []   # (out_ps, y1n, pair, q_sl) awaiting normalize

            def flush_mul():
                while pending_mul:
                    out_ps, y1n, ppair, psl = pending_mul.pop(0)
                    o_sb = out_pool.tile([P, QBLK], f32, tag="osb")
                    nc.vector.tensor_mul(o_sb[:], out_ps[:], y1n[:])
                    nc.sync.dma_start(outT[ppair][:, psl], o_sb[:])

            def flush_sums():
                while pending_sums:
                    acc, out_ps, ppair, psl = pending_sums.pop(0)
                    # partition-reduce both acc halves into one PSUM bank
                    # (2-matmul accumulation; no separate fold op needed);
                    # the sums tile squats in the sc ring, keeping PSUM at
                    # exactly 8 banks with triple-buffered scores
                    sums_ps = sc_psum.tile([P, GW], f32, tag="sc")
                    nc.tensor.matmul(
                        sums_ps[:, :QBLK], ones[:], acc[:, :QBLK],
                        start=True, stop=False,
                    )
                    nc.tensor.matmul(
                        sums_ps[:, :QBLK], ones[:], acc[:, QBLK:],
                        start=False, stop=True,
                    )
                    sums_sb = norm_pool.tile([P, QBLK], f32, tag="sums_sb")
                    nc.vector.tensor_copy(sums_sb[:], sums_ps[:, :QBLK])
                    # fast inverse: bit-trick seed + 1 Newton step on GPSIMD
                    y0i = norm_pool.tile([P, QBLK], i32, tag="y0")
                    nc.gpsimd.tensor_scalar(
                        y0i[:], sums_sb[:].bitcast(i32), -1, RECIP_MAGIC,
                        mult, add,
                    )
                    y0f = y0i[:].bitcast(f32)
                    t2 = norm_pool.tile([P, QBLK], f32, tag="t2")
                    nc.gpsimd.tensor_tensor(t2[:], sums_sb[:], y0f, mult)
                    u2 = norm_pool.tile([P, QBLK], f32, tag="u2")
                    nc.gpsimd.tensor_scalar(
                        u2[:], t2[:], -1.0, 2.0, mult, add
                    )
                    y1n = norm_pool.tile([P, QBLK], f32, tag="y1n")
                    nc.gpsimd.tensor_tensor(y1n[:], u2[:], y0f, mult)
                    pending_mul.append((out_ps, y1n, ppair, psl))

            # One flat software pipeline over all (pair, qb, g) groups with a
            # constant 2-group PV lag: a group's PV matmuls are emitted two
            # group-slots after its scores, so the ~1.1us exp latency is
            # always covered and q-block boundaries never bubble (the next
            # q-block's scores are emitted before this one's last PVs).
            pv_queue = []  # (g, ef, out_ps, v_sb, start, stop)

            def emit_pv_lagged(drain=False):
                while len(pv_queue) > (0 if drain else 2):
                    g, ef, ops, vsb, st, sp = pv_queue.pop(0)
                    for j in range(SK_PER_GROUP):
                        sk = g * SK_PER_GROUP + j
                        nc.tensor.matmul(
                            ops[:],
                            vsb[:, sk, :],
                            ef[:, j * QBLK : (j + 1) * QBLK],
                            start=(st and j == 0),
                            stop=(sp and j == SK_PER_GROUP - 1),
                        )

            # software prefetch: emit the next pair's load DMAs before the
            # current pair's compute so transfers fully overlap it
            cur_tiles = emit_loads(0)
            for pair in range(PAIRS_PER_CORE):
                qT_sb, kT_sb, v_sb = cur_tiles
                if pair + 1 < PAIRS_PER_CORE:
                    cur_tiles = emit_loads(pair + 1)

                for qb in range(N_QB):
                    q_sl = slice(qb * QBLK, (qb + 1) * QBLK)
                    out_ps = o_psum.tile([P, QBLK], f32, tag="ops")
                    acc = acc_pool.tile([P, GW], f16, tag="acc")

                    e_tiles = [None] * N_GROUPS
                    for g in range(N_GROUPS):
                        sc = sc_psum.tile([P, GW], f32, tag="sc")
                        for j in range(SK_PER_GROUP):
                            sk = g * SK_PER_GROUP + j
                            nc.tensor.matmul(
                                sc[:, j * QBLK : (j + 1) * QBLK],
                                kT_sb[:, sk * P : (sk + 1) * P],
                                qT_sb[:, q_sl],
                                start=True,
                                stop=True,
                            )
                        ei = exp_pool.tile([P, GW], i16, tag="e")
                        ef = ei[:].bitcast(f16)
                        if g == OFF_GROUP:
                            # Schraudolph: fp16 exp bits via one tensor_scalar
                            nc.vector.tensor_scalar(
                                ei[:], sc[:], EXP_A * SCALE, EXP_B,
                                mult, add,
                            )
                        else:
                            nc.scalar.activation(
                                ef, sc[:],
                                mybir.ActivationFunctionType.Exp,
                                scale=SCALE,
                            )
                        e_tiles[g] = ef
                        if g == 1:
                            # normalize+store for q-block i-2 (its 1/sums
                            # finished on GPSIMD during q-block i-1)
                            flush_mul()
                        if g == 3:
                            # previous q-block's sums chain: emitted here so
                            # its acc dependency (complete shortly past the
                            # boundary) never stalls the PE stream
                            flush_sums()
                        # softmax-sum adds strictly in group order (the
                        # OFF_GROUP add follows its DVE exp immediately), so
                        # acc completes one add after the last ACT exp
                        if g == 1:
                            nc.vector.tensor_add(
                                acc[:], e_tiles[0], e_tiles[1]
                            )
                        elif g > 1:
                            nc.vector.tensor_add(acc[:], acc[:], ef)
                        pv_queue.append(
                            (g, ef, out_ps, v_sb,
                             g == 0, g == N_GROUPS - 1)
                        )
                        emit_pv_lagged()
                    pending_sums.append((acc, out_ps, pair, q_sl))

            emit_pv_lagged(drain=True)
            flush_sums()
            flush_mul()

    _split_excess_waits(nc)
    return nc


def _get_compiled():
    global _COMPILED
    if _COMPILED is None:
        _COMPILED = _build()
    return _COMPILED


def _round_f32r(x):
    """Round fp32 to the fp32r grid: round-to-nearest-even at 11 mantissa
    bits (verified bit-exact against the on-chip DVE fp32->fp32r cast)."""
    b = np.ascontiguousarray(x).view(np.uint32).astype(np.uint64)
    drop = np.uint64(12)
    half = np.uint64(1 << 11)
    lsb = (b >> drop) & np.uint64(1)
    r = (b + half - np.uint64(1) + lsb) & np.uint64(0xFFFFF000)
    return r.astype(np.uint32).view(np.float32).reshape(x.shape)


def _shard_inputs(query, key, value):
    """Full [B,S,H,D] inputs -> per-core input maps (host-side Ulysses)."""
    # [B,S,H,D] -> [B,H,D,S] -> [BH, D, S] for q/k; [B,H,S,D] -> [BH, S, D] for v
    qT_all = np.ascontiguousarray(np.transpose(query, (0, 2, 3, 1))).reshape(
        B * H, D, S
    )
    kT_all = np.ascontiguousarray(np.transpose(key, (0, 2, 3, 1))).reshape(
        B * H, D, S
    )
    v_all = np.ascontiguousarray(np.transpose(value, (0, 2, 1, 3))).reshape(
        B * H, S, D
    )
    in_maps = []
    for c in range(N_CORES):
        sl = slice(c * PAIRS_PER_CORE, (c + 1) * PAIRS_PER_CORE)
        in_maps.append(
            {
                "qT": _round_f32r(qT_all[sl]),
                "kT": _round_f32r(kT_all[sl]),
                "v": np.ascontiguousarray(v_all[sl]).astype(np.float16),
            }
        )
    return in_maps


def _gather_output(results):
    outT_all = np.concatenate([r["outT"] for r in results], axis=0)  # [BH, D, S]
    out = outT_all.reshape(B, H, D, S).transpose(0, 3, 1, 2)  # [B, S, H, D]
    return np.ascontiguousarray(out)


def kernel(query, key, value, _run_kwargs=None):
    from concourse.bass_utils import run_bass_kernel_spmd

    nc = _get_compiled()
    in_maps = _shard_inputs(
        np.asarray(query, dtype=np.float32),
        np.asarray(key, dtype=np.float32),
        np.asarray(value, dtype=np.float32),
    )
    kwargs = _run_kwargs or {}
    res = run_bass_kernel_spmd(nc, in_maps, core_ids=list(range(N_CORES)), **kwargs)
    out = _gather_output(res.results)
    if _run_kwargs is not None:
        kernel.last_result = res
    return out


# revision 15
# speedup vs baseline: 1.2105x; 1.0164x over previous
"""Multi-head attention (B=2, S=2048, H=16, D=128, fp32, non-causal) on 8
Trainium2 NeuronCores.

Strategy: the 32 (batch, head) pairs are independent -> head-parallel
(Ulysses-style) sharding, 4 pairs per core, no on-device collectives.
The host pre-transposes Q and K to [d, s] layout per pair (so the
contraction dim d lands on SBUF partitions with no on-chip transposes),
and the kernel produces out^T [d, s] which the host transposes back.

v2 engine balance (the v1 kernel was ACT-bound at 93% busy):
- exp of scores is split between the ACT engine (table exp, fp16 out) and
  the DVE via a Schraudolph bit-trick: one tensor_scalar computing
  round(score * SCALE*1024/ln2 + 15316) into an int16 tile whose bits ARE
  the fp16 exp approximation (max rel err ~3%, which perturbs the softmax
  output by <1e-3 of its max; conversion rounding verified on HW).
- softmax reciprocal moved off ACT entirely: 1/sums via a fast-inverse
  int32 bit-trick seed + one Newton step, all on the otherwise-idle GPSIMD
  engine (stock tensor ops; ~1.2e-3 rel err).
- the final normalize multiply is emitted one q-block late so the
  in-order DVE/GPSIMD queues never stall the steady-state exp pipeline.
- ACT runs a pure exp stream (no Ln/Exp reciprocal chain, no stalls).
"""

import math

import numpy as np

B, S, H, D = 2, 2048, 16, 128
N_CORES = 8
PAIRS_PER_CORE = (B * H) // N_CORES  # 4
P = 128
QBLK = 512  # q columns per q-block (one PSUM bank of fp32)
N_QB = S // QBLK  # 4
N_SK = S // P  # 16 sk tiles per pair
SK_PER_GROUP = 2  # sk tiles per scores/exp group ([128, 1024] psum tiles)
N_GROUPS = N_SK // SK_PER_GROUP  # 8
GW = SK_PER_GROUP * QBLK  # group width: 1024
SCALE = 1.0 / math.sqrt(D)

# group whose exp runs on DVE instead of ACT (None = all ACT). Its PV
# matmuls and softmax-sum add are deferred to the end of the q-block, so the
# DVE latency hides behind the other 7 groups' compute. Group 3 is chosen so
# the DVE op sits mid-stream, clear of the q-block-boundary critical path.
OFF_GROUP = 3

# Schraudolph fp16 exp: bits16 = round(x * EXP_A + EXP_B); bitcast -> fp16
EXP_A = 1024.0 / math.log(2.0)
EXP_B = 15360.0 - 44.0
# fast inverse seed: y0_bits = RECIP_MAGIC - bits(x)
RECIP_MAGIC = 0x7EF311C3

_COMPILED = None


def _patch_tile_drain():
    """Workaround for walrus 'Too many sync wait commands' on the TileContext
    tail Drain: redistribute all but one of the drain's sem waits onto
    single-wait NoOps on the sync engine (program order places them after the
    drain and before the all-engine barrier, which preserves semantics)."""
    import concourse.mybir as mybir
    import concourse.tile as tile
    from concourse.vector_clock import ScopedClock

    if getattr(tile.TileContext, "_ant_drain_patched", False):
        return

    def _drain_and_barrier(self, tick_clock, wait_clock):
        drain_inst = self.nc.sync.drain()
        wait_clock.add_sem_waits(
            drain_inst.ins, ScopedClock({None: tick_clock.global_clock})
        )
        si = drain_inst.ins.sync_info
        if si is not None and si.on_wait and len(si.on_wait) > 1:
            waits = list(si.on_wait)
            si.on_wait = waits[:1]
            # distribute the remaining waits round-robin across engines so
            # they are honored in parallel; the all-engine barrier below
            # collects them all before the semaphore reset
            engines = [
                self.nc.sync, self.nc.vector, self.nc.scalar,
                self.nc.tensor, self.nc.gpsimd,
            ]
            for i, w in enumerate(waits[1:]):
                nop = engines[i % len(engines)].nop(nofuse=True)
                nop.ins.sync_info = mybir.SyncInfo(on_wait=[w], on_update=[])

        self.nc.all_engine_barrier()
        assert self.sems is not None
        popped = self.nc._tile_sem_poison_stack.pop()
        assert popped is self._sem_poison
        self.nc.clear_and_free_semaphores(list(self.sems.allocated().values()))
        self.nc.all_engine_barrier()

    tile.TileContext._drain_and_barrier = _drain_and_barrier
    tile.TileContext._ant_drain_patched = True


def _split_excess_waits(nc):
    """This container's walrus rejects instructions carrying more than a
    struct-dependent number of semaphore waits (setupSyncWait: 'Too many
    sync wait commands'): 1 for Matmult/Ldweights (S3_LW struct), 2 for
    everything else. Hoist the excess onto NoOps inserted just before the
    instruction on the same engine — same-engine program order guarantees
    they are honored before the instruction issues."""
    import concourse.mybir as mybir

    seq = 0
    for f in nc.m.functions:
        for b in f.blocks:
            insts = list(b.instructions)
            out = []
            changed = False
            for inst in insts:
                max_waits = 1
                si = inst.sync_info
                if si is not None and si.on_wait and len(si.on_wait) > max_waits:
                    waits = list(si.on_wait)
                    si.on_wait = waits[:max_waits]
                    # NoOps (CTRL struct) only take 1 wait each
                    for w in waits[max_waits:]:
                        nop = mybir.InstNoOp(name=f"ant-waitsplit-{seq}")
                        seq += 1
                        nop.engine = inst.engine
                        nop.sync_info = mybir.SyncInfo(
                            on_wait=[w], on_update=[]
                        )
                        out.append(nop)
                    changed = True
                out.append(inst)
            if changed:
                b.instructions = out
    return nc


def _build():
    import concourse.bass as bass
    import concourse.mybir as mybir
    import concourse.tile as tile

    _patch_tile_drain()

    f32 = mybir.dt.float32
    f32r = mybir.dt.float32r
    f16 = mybir.dt.float16
    i16 = mybir.dt.int16
    i32 = mybir.dt.int32
    mult = mybir.AluOpType.mult
    add = mybir.AluOpType.add
    subtract = mybir.AluOpType.subtract
    nc = bass.Bass()

    # Q/K arrive pre-rounded to the fp32r grid (RNE at 11 mantissa bits,
    # verified bit-exact against the on-chip DVE cast) so they DMA straight
    # into fp32r tiles; V arrives pre-cast to fp16.
    qT = nc.dram_tensor("qT", [PAIRS_PER_CORE, P, S], f32r, kind="ExternalInput")
    kT = nc.dram_tensor("kT", [PAIRS_PER_CORE, P, S], f32r, kind="ExternalInput")
    v = nc.dram_tensor("v", [PAIRS_PER_CORE, S, D], f16, kind="ExternalInput")
    outT = nc.dram_tensor("outT", [PAIRS_PER_CORE, P, S], f32, kind="ExternalOutput")

    with tile.TileContext(nc) as tc:
        with (
            tc.tile_pool(name="const", bufs=1) as const_pool,
            tc.tile_pool(name="inp", bufs=2) as inp_pool,
            tc.tile_pool(name="exp", bufs=8) as exp_pool,
            tc.tile_pool(name="acc", bufs=2) as acc_pool,
            tc.tile_pool(name="norm", bufs=2) as norm_pool,
            tc.tile_pool(name="outsb", bufs=3) as out_pool,
            tc.tile_pool(name="sc_ps", bufs=3, space="PSUM") as sc_psum,
            tc.tile_pool(name="o_ps", bufs=2, space="PSUM") as o_psum,
        ):
            ones_ld = const_pool.tile([P, P], f32)
            nc.vector.memset(ones_ld[:], 1.0)
            ones = const_pool.tile([P, P], f16)
            nc.vector.tensor_copy(ones[:], ones_ld[:])

            def emit_loads(pair):
                # chunked so the first scores matmuls start sooner: the
                # first q-block needs qT[:, :512] and kT tiles in order
                qT_sb = inp_pool.tile([P, S], f32r, tag="qT")
                kT_sb = inp_pool.tile([P, S], f32r, tag="kT")
                v_sb = inp_pool.tile([P, N_SK, D], f16, tag="v")
                nQ = 4
                for h in range(nQ):
                    sl = slice(h * (S // nQ), (h + 1) * (S // nQ))
                    nc.sync.dma_start(kT_sb[:, sl], kT[pair][:, sl])
                    if h == 0:
                        nc.sync.dma_start(qT_sb[:, sl], qT[pair][:, sl])
                rest = slice(S // nQ, S)
                nc.sync.dma_start(qT_sb[:, rest], qT[pair][:, rest])
                nc.sync.dma_start(
                    v_sb[:], v[pair].rearrange("(t p) d -> p t d", p=P)
                )
                return qT_sb, kT_sb, v_sb

            # pending normalize from the previous q-block:
            # (out_ps tile, -1/sums tile, pair idx, q slice)
            # deferred-by-one-q-block pipelines: nothing dependency-heavy is
            # ever emitted at the tail of a q-block, so no engine's in-order
            # queue blocks the next q-block's scores/exp stream.
            pending_sums = []  # (acc, out_ps, pair, q_sl) awaiting sums chain
            pending_mul = []   # (out_ps, y1n, pair, q_sl) awaiting normalize

            def flush_mul():
                while pending_mul:
                    out_ps, y1n, ppair, psl = pending_mul.pop(0)
                    o_sb = out_pool.tile([P, QBLK], f32, tag="osb")
                    nc.vector.tensor_mul(o_sb[:], out_ps[:], y1n[:])
                    nc.sync.dma_start(outT[ppair][:, psl], o_sb[:])

            def flush_sums():
                while pending_sums:
                    acc, out_ps, ppair, psl = pending_sums.pop(0)
                    # partition-reduce both acc halves into one PSUM bank
                    # (2-matmul accumulation; no separate fold op needed);
                    # the sums tile squats in the sc ring, keeping PSUM at
                    # exactly 8 banks with triple-buffered scores
                    sums_ps = sc_psum.tile([P, GW], f32, tag="sc")
                    nc.tensor.matmul(
                        sums_ps[:, :QBLK], ones[:], acc[:, :QBLK],
                        start=True, stop=False,
                    )
                    nc.tensor.matmul(
                        sums_ps[:, :QBLK], ones[:], acc[:, QBLK:],
                        start=False, stop=True,
                    )
                    sums_sb = norm_pool.tile([P, QBLK], f32, tag="sums_sb")
                    nc.vector.tensor_copy(sums_sb[:], sums_ps[:, :QBLK])
                    # fast inverse: bit-trick seed + 1 Newton step on GPSIMD
                    y0i = norm_pool.tile([P, QBLK], i32, tag="y0")
                    nc.gpsimd.tensor_scalar(
                        y0i[:], sums_sb[:].bitcast(i32), -1, RECIP_MAGIC,
                        mult, add,
                    )
                    y0f = y0i[:].bitcast(f32)
                    t2 = norm_pool.tile([P, QBLK], f32, tag="t2")
                    nc.gpsimd.tensor_tensor(t2[:], sums_sb[:], y0f, mult)
                    u2 = norm_pool.tile([P, QBLK], f32, tag="u2")
                    nc.gpsimd.tensor_scalar(
                        u2[:], t2[:], -1.0, 2.0, mult, add
                    )
                    y1n = norm_pool.tile([P, QBLK], f32, tag="y1n")
                    nc.gpsimd.tensor_tensor(y1n[:], u2[:], y0f, mult)
                    pending_mul.append((out_ps, y1n, ppair, psl))

            # One flat software pipeline over all (pair, qb, g) groups with a
            # constant 2-group PV lag: a group's PV matmuls are emitted two
            # group-slots after its scores, so the ~1.1us exp latency is
            # always covered and q-block boundaries never bubble (the next
            # q-block's scores are emitted before this one's last PVs).
            pv_queue = []  # (g, ef, out_ps, v_sb, start, stop)

            def emit_pv_lagged(drain=False):
                while len(pv_queue) > (0 if drain else 4):
                    g, ef, ops, vsb, st, sp = pv_queue.pop(0)
                    for j in range(SK_PER_GROUP):
                        sk = g * SK_PER_GROUP + j
                        nc.tensor.matmul(
                            ops[:],
                            vsb[:, sk, :],
                            ef[:, j * QBLK : (j + 1) * QBLK],
                            start=(st and j == 0),
                            stop=(sp and j == SK_PER_GROUP - 1),
                        )

            # software prefetch: emit the next pair's load DMAs before the
            # current pair's compute so transfers fully overlap it
            cur_tiles = emit_loads(0)
            for pair in range(PAIRS_PER_CORE):
                qT_sb, kT_sb, v_sb = cur_tiles
                if pair + 1 < PAIRS_PER_CORE:
                    cur_tiles = emit_loads(pair + 1)

                for qb in range(N_QB):
                    q_sl = slice(qb * QBLK, (qb + 1) * QBLK)
                    out_ps = o_psum.tile([P, QBLK], f32, tag="ops")
                    acc = acc_pool.tile([P, GW], f16, tag="acc")

                    e_tiles = [None] * N_GROUPS
                    for g in range(N_GROUPS):
                        sc = sc_psum.tile([P, GW], f32, tag="sc")
                        for j in range(SK_PER_GROUP):
                            sk = g * SK_PER_GROUP + j
                            nc.tensor.matmul(
                                sc[:, j * QBLK : (j + 1) * QBLK],
                                kT_sb[:, sk * P : (sk + 1) * P],
                                qT_sb[:, q_sl],
                                start=True,
                                stop=True,
                            )
                        ei = exp_pool.tile([P, GW], i16, tag="e")
                        ef = ei[:].bitcast(f16)
                        if g == OFF_GROUP:
                            # Schraudolph: fp16 exp bits via one tensor_scalar
                            nc.vector.tensor_scalar(
                                ei[:], sc[:], EXP_A * SCALE, EXP_B,
                                mult, add,
                            )
                        else:
                            nc.scalar.activation(
                                ef, sc[:],
                                mybir.ActivationFunctionType.Exp,
                                scale=SCALE,
                            )
                        e_tiles[g] = ef
                        if g == 1:
                            # normalize+store for q-block i-2 (its 1/sums
                            # finished on GPSIMD during q-block i-1)
                            flush_mul()
                        if g == 3:
                            # previous q-block's sums chain: emitted here so
                            # its acc dependency (complete shortly past the
                            # boundary) never stalls the PE stream
                            flush_sums()
                        # softmax-sum adds strictly in group order (the
                        # OFF_GROUP add follows its DVE exp immediately), so
                        # acc completes one add after the last ACT exp
                        if g == 1:
                            nc.vector.tensor_add(
                                acc[:], e_tiles[0], e_tiles[1]
                            )
                        elif g > 1:
                            nc.vector.tensor_add(acc[:], acc[:], ef)
                        pv_queue.append(
                            (g, ef, out_ps, v_sb,
                             g == 0, g == N_GROUPS - 1)
                        )
                        emit_pv_lagged()
                    pending_sums.append((acc, out_ps, pair, q_sl))

            emit_pv_lagged(drain=True)
            flush_sums()
            flush_mul()

    _split_excess_waits(nc)
    return nc


def _get_compiled():
    global _COMPILED
    if _COMPILED is None:
        _COMPILED = _build()
    return _COMPILED


def _round_f32r(x):
    """Round fp32 to the fp32r grid: round-to-nearest-even at 11 mantissa
    bits (verified bit-exact against the on-chip DVE fp32->fp32r cast)."""
    b = np.ascontiguousarray(x).view(np.uint32).astype(np.uint64)
    drop = np.uint64(12)
    half = np.uint64(1 << 11)
    lsb = (b >> drop) & np.uint64(1)
    r = (b + half - np.uint64(1) + lsb) & np.uint64(0xFFFFF000)
    return r.astype(np.uint32).view(np.float32).reshape(x.shape)


def _shard_inputs(query, key, value):
    """Full [B,S,H,D] inputs -> per-core input maps (host-side Ulysses)."""
    # [B,S,H,D] -> [B,H,D,S] -> [BH, D, S] for q/k; [B,H,S,D] -> [BH, S, D] for v
    qT_all = np.ascontiguousarray(np.transpose(query, (0, 2, 3, 1))).reshape(
        B * H, D, S
    )
    kT_all = np.ascontiguousarray(np.transpose(key, (0, 2, 3, 1))).reshape(
        B * H, D, S
    )
    v_all = np.ascontiguousarray(np.transpose(value, (0, 2, 1, 3))).reshape(
        B * H, S, D
    )
    in_maps = []
    for c in range(N_CORES):
        sl = slice(c * PAIRS_PER_CORE, (c + 1) * PAIRS_PER_CORE)
        in_maps.append(
            {
                "qT": _round_f32r(qT_all[sl]),
                "kT": _round_f32r(kT_all[sl]),
                "v": np.ascontiguousarray(v_all[sl]).astype(np.float16),
            }
        )
    return in_maps


def _gather_output(results):
    outT_all = np.concatenate([r["outT"] for r in results], axis=0)  # [BH, D, S]
    out = outT_all.reshape(B, H, D, S).transpose(0, 3, 1, 2)  # [B, S, H, D]
    return np.ascontiguousarray(out)


def kernel(query, key, value, _run_kwargs=None):
    from concourse.bass_utils import run_bass_kernel_spmd

    nc = _get_compiled()
    in_maps = _shard_inputs(
        np.asarray(query, dtype=np.float32),
        np.asarray(key, dtype=np.float32),
        np.asarray(value, dtype=np.float32),
    )
    kwargs = _run_kwargs or {}
    res = run_bass_kernel_spmd(nc, in_maps, core_ids=list(range(N_CORES)), **kwargs)
    out = _gather_output(res.results)
    if _run_kwargs is not None:
        kernel.last_result = res
    return out


# revision 16
# speedup vs baseline: 1.2115x; 1.0009x over previous
"""Multi-head attention (B=2, S=2048, H=16, D=128, fp32, non-causal) on 8
Trainium2 NeuronCores.

Strategy: the 32 (batch, head) pairs are independent -> head-parallel
(Ulysses-style) sharding, 4 pairs per core, no on-device collectives.
The host pre-transposes Q and K to [d, s] layout per pair (so the
contraction dim d lands on SBUF partitions with no on-chip transposes),
and the kernel produces out^T [d, s] which the host transposes back.

v2 engine balance (the v1 kernel was ACT-bound at 93% busy):
- exp of scores is split between the ACT engine (table exp, fp16 out) and
  the DVE via a Schraudolph bit-trick: one tensor_scalar computing
  round(score * SCALE*1024/ln2 + 15316) into an int16 tile whose bits ARE
  the fp16 exp approximation (max rel err ~3%, which perturbs the softmax
  output by <1e-3 of its max; conversion rounding verified on HW).
- softmax reciprocal moved off ACT entirely: 1/sums via a fast-inverse
  int32 bit-trick seed + one Newton step, all on the otherwise-idle GPSIMD
  engine (stock tensor ops; ~1.2e-3 rel err).
- the final normalize multiply is emitted one q-block late so the
  in-order DVE/GPSIMD queues never stall the steady-state exp pipeline.
- ACT runs a pure exp stream (no Ln/Exp reciprocal chain, no stalls).
"""

import math

import numpy as np

B, S, H, D = 2, 2048, 16, 128
N_CORES = 8
PAIRS_PER_CORE = (B * H) // N_CORES  # 4
P = 128
QBLK = 512  # q columns per q-block (one PSUM bank of fp32)
N_QB = S // QBLK  # 4
N_SK = S // P  # 16 sk tiles per pair
SK_PER_GROUP = 2  # sk tiles per scores/exp group ([128, 1024] psum tiles)
N_GROUPS = N_SK // SK_PER_GROUP  # 8
GW = SK_PER_GROUP * QBLK  # group width: 1024
SCALE = 1.0 / math.sqrt(D)

# group whose exp runs on DVE instead of ACT (None = all ACT). Its PV
# matmuls and softmax-sum add are deferred to the end of the q-block, so the
# DVE latency hides behind the other 7 groups' compute. Group 3 is chosen so
# the DVE op sits mid-stream, clear of the q-block-boundary critical path.
OFF_GROUP = 3

# Schraudolph fp16 exp: bits16 = round(x * EXP_A + EXP_B); bitcast -> fp16
EXP_A = 1024.0 / math.log(2.0)
EXP_B = 15360.0 - 44.0
# fast inverse seed: y0_bits = RECIP_MAGIC - bits(x)
RECIP_MAGIC = 0x7EF311C3

_COMPILED = None


def _patch_tile_drain():
    """Workaround for walrus 'Too many sync wait commands' on the TileContext
    tail Drain: redistribute all but one of the drain's sem waits onto
    single-wait NoOps on the sync engine (program order places them after the
    drain and before the all-engine barrier, which preserves semantics)."""
    import concourse.mybir as mybir
    import concourse.tile as tile
    from concourse.vector_clock import ScopedClock

    if getattr(tile.TileContext, "_ant_drain_patched", False):
        return

    def _drain_and_barrier(self, tick_clock, wait_clock):
        drain_inst = self.nc.sync.drain()
        wait_clock.add_sem_waits(
            drain_inst.ins, ScopedClock({None: tick_clock.global_clock})
        )
        si = drain_inst.ins.sync_info
        if si is not None and si.on_wait and len(si.on_wait) > 1:
            waits = list(si.on_wait)
            si.on_wait = waits[:1]
            # distribute the remaining waits round-robin across engines so
            # they are honored in parallel; the all-engine barrier below
            # collects them all before the semaphore reset
            engines = [
                self.nc.sync, self.nc.vector, self.nc.scalar,
                self.nc.tensor, self.nc.gpsimd,
            ]
            for i, w in enumerate(waits[1:]):
                nop = engines[i % len(engines)].nop(nofuse=True)
                nop.ins.sync_info = mybir.SyncInfo(on_wait=[w], on_update=[])

        self.nc.all_engine_barrier()
        assert self.sems is not None
        popped = self.nc._tile_sem_poison_stack.pop()
        assert popped is self._sem_poison
        self.nc.clear_and_free_semaphores(list(self.sems.allocated().values()))
        self.nc.all_engine_barrier()

    tile.TileContext._drain_and_barrier = _drain_and_barrier
    tile.TileContext._ant_drain_patched = True


def _split_excess_waits(nc):
    """This container's walrus rejects instructions carrying more than a
    struct-dependent number of semaphore waits (setupSyncWait: 'Too many
    sync wait commands'): 1 for Matmult/Ldweights (S3_LW struct), 2 for
    everything else. Hoist the excess onto NoOps inserted just before the
    instruction on the same engine — same-engine program order guarantees
    they are honored before the instruction issues."""
    import concourse.mybir as mybir

    seq = 0
    for f in nc.m.functions:
        for b in f.blocks:
            insts = list(b.instructions)
            out = []
            changed = False
            for inst in insts:
                max_waits = 1
                si = inst.sync_info
                if si is not None and si.on_wait and len(si.on_wait) > max_waits:
                    waits = list(si.on_wait)
                    si.on_wait = waits[:max_waits]
                    # NoOps (CTRL struct) only take 1 wait each
                    for w in waits[max_waits:]:
                        nop = mybir.InstNoOp(name=f"ant-waitsplit-{seq}")
                        seq += 1
                        nop.engine = inst.engine
                        nop.sync_info = mybir.SyncInfo(
                            on_wait=[w], on_update=[]
                        )
                        out.append(nop)
                    changed = True
                out.append(inst)
            if changed:
                b.instructions = out
    return nc


def _build():
    import concourse.bass as bass
    import concourse.mybir as mybir
    import concourse.tile as tile

    _patch_tile_drain()

    f32 = mybir.dt.float32
    f32r = mybir.dt.float32r
    f16 = mybir.dt.float16
    i16 = mybir.dt.int16
    i32 = mybir.dt.int32
    mult = mybir.AluOpType.mult
    add = mybir.AluOpType.add
    subtract = mybir.AluOpType.subtract
    nc = bass.Bass()

    # Q/K/V all arrive pre-cast to fp16 so every matmul runs in one uniform
    # PE mode (alternating fp32r/fp16 stationary modes measurably stalls the
    # PE pipeline), with half the DMA traffic. Scores quantization error from
    # fp16 Q/K is ~3e-4 -- far below the exp-approximation error budget.
    qT = nc.dram_tensor("qT", [PAIRS_PER_CORE, P, S], f16, kind="ExternalInput")
    kT = nc.dram_tensor("kT", [PAIRS_PER_CORE, P, S], f16, kind="ExternalInput")
    v = nc.dram_tensor("v", [PAIRS_PER_CORE, S, D], f16, kind="ExternalInput")
    outT = nc.dram_tensor("outT", [PAIRS_PER_CORE, P, S], f32, kind="ExternalOutput")

    with tile.TileContext(nc) as tc:
        with (
            tc.tile_pool(name="const", bufs=1) as const_pool,
            tc.tile_pool(name="inp", bufs=2) as inp_pool,
            tc.tile_pool(name="exp", bufs=8) as exp_pool,
            tc.tile_pool(name="acc", bufs=2) as acc_pool,
            tc.tile_pool(name="norm", bufs=2) as norm_pool,
            tc.tile_pool(name="outsb", bufs=3) as out_pool,
            tc.tile_pool(name="sc_ps", bufs=3, space="PSUM") as sc_psum,
            tc.tile_pool(name="o_ps", bufs=2, space="PSUM") as o_psum,
        ):
            ones_ld = const_pool.tile([P, P], f32)
            nc.vector.memset(ones_ld[:], 1.0)
            ones = const_pool.tile([P, P], f16)
            nc.vector.tensor_copy(ones[:], ones_ld[:])

            def emit_loads(pair):
                # chunked so the first scores matmuls start sooner: the
                # first q-block needs qT[:, :512] and kT tiles in order
                qT_sb = inp_pool.tile([P, S], f16, tag="qT")
                kT_sb = inp_pool.tile([P, S], f16, tag="kT")
                v_sb = inp_pool.tile([P, N_SK, D], f16, tag="v")
                nQ = 4
                for h in range(nQ):
                    sl = slice(h * (S // nQ), (h + 1) * (S // nQ))
                    nc.sync.dma_start(kT_sb[:, sl], kT[pair][:, sl])
                    if h == 0:
                        nc.sync.dma_start(qT_sb[:, sl], qT[pair][:, sl])
                rest = slice(S // nQ, S)
                nc.sync.dma_start(qT_sb[:, rest], qT[pair][:, rest])
                nc.sync.dma_start(
                    v_sb[:], v[pair].rearrange("(t p) d -> p t d", p=P)
                )
                return qT_sb, kT_sb, v_sb

            # pending normalize from the previous q-block:
            # (out_ps tile, -1/sums tile, pair idx, q slice)
            # deferred-by-one-q-block pipelines: nothing dependency-heavy is
            # ever emitted at the tail of a q-block, so no engine's in-order
            # queue blocks the next q-block's scores/exp stream.
            pending_sums = []  # (acc, out_ps, pair, q_sl) awaiting sums chain
            pending_mul = []   # (out_ps, y1n, pair, q_sl) awaiting normalize

            def flush_mul():
                while pending_mul:
                    out_ps, y1n, ppair, psl = pending_mul.pop(0)
                    o_sb = out_pool.tile([P, QBLK], f32, tag="osb")
                    nc.vector.tensor_mul(o_sb[:], out_ps[:], y1n[:])
                    nc.sync.dma_start(outT[ppair][:, psl], o_sb[:])

            def flush_sums():
                while pending_sums:
                    acc, out_ps, ppair, psl = pending_sums.pop(0)
                    # partition-reduce both acc halves into one PSUM bank
                    # (2-matmul accumulation; no separate fold op needed);
                    # the sums tile squats in the sc ring, keeping PSUM at
                    # exactly 8 banks with triple-buffered scores
                    sums_ps = sc_psum.tile([P, GW], f32, tag="sc")
                    nc.tensor.matmul(
                        sums_ps[:, :QBLK], ones[:], acc[:, :QBLK],
                        start=True, stop=False,
                    )
                    nc.tensor.matmul(
                        sums_ps[:, :QBLK], ones[:], acc[:, QBLK:],
                        start=False, stop=True,
                    )
                    sums_sb = norm_pool.tile([P, QBLK], f32, tag="sums_sb")
                    nc.vector.tensor_copy(sums_sb[:], sums_ps[:, :QBLK])
                    # fast inverse: bit-trick seed + 1 Newton step on GPSIMD
                    y0i = norm_pool.tile([P, QBLK], i32, tag="y0")
                    nc.gpsimd.tensor_scalar(
                        y0i[:], sums_sb[:].bitcast(i32), -1, RECIP_MAGIC,
                        mult, add,
                    )
                    y0f = y0i[:].bitcast(f32)
                    t2 = norm_pool.tile([P, QBLK], f32, tag="t2")
                    nc.gpsimd.tensor_tensor(t2[:], sums_sb[:], y0f, mult)
                    u2 = norm_pool.tile([P, QBLK], f32, tag="u2")
                    nc.gpsimd.tensor_scalar(
                        u2[:], t2[:], -1.0, 2.0, mult, add
                    )
                    y1n = norm_pool.tile([P, QBLK], f32, tag="y1n")
                    nc.gpsimd.tensor_tensor(y1n[:], u2[:], y0f, mult)
                    pending_mul.append((out_ps, y1n, ppair, psl))

            # One flat software pipeline over all (pair, qb, g) groups with a
            # constant 2-group PV lag: a group's PV matmuls are emitted two
            # group-slots after its scores, so the ~1.1us exp latency is
            # always covered and q-block boundaries never bubble (the next
            # q-block's scores are emitted before this one's last PVs).
            pv_queue = []  # (g, ef, out_ps, v_sb, start, stop)

            def emit_pv_lagged(drain=False):
                while len(pv_queue) > (0 if drain else 4):
                    g, ef, ops, vsb, st, sp = pv_queue.pop(0)
                    for j in range(SK_PER_GROUP):
                        sk = g * SK_PER_GROUP + j
                        nc.tensor.matmul(
                            ops[:],
                            vsb[:, sk, :],
                            ef[:, j * QBLK : (j + 1) * QBLK],
                            start=(st and j == 0),
                            stop=(sp and j == SK_PER_GROUP - 1),
                        )

            # software prefetch: emit the next pair's load DMAs before the
            # current pair's compute so transfers fully overlap it
            cur_tiles = emit_loads(0)
            for pair in range(PAIRS_PER_CORE):
                qT_sb, kT_sb, v_sb = cur_tiles
                if pair + 1 < PAIRS_PER_CORE:
                    cur_tiles = emit_loads(pair + 1)

                for qb in range(N_QB):
                    q_sl = slice(qb * QBLK, (qb + 1) * QBLK)
                    out_ps = o_psum.tile([P, QBLK], f32, tag="ops")
                    acc = acc_pool.tile([P, GW], f16, tag="acc")

                    e_tiles = [None] * N_GROUPS
                    for g in range(N_GROUPS):
                        sc = sc_psum.tile([P, GW], f32, tag="sc")
                        for j in range(SK_PER_GROUP):
                            sk = g * SK_PER_GROUP + j
                            nc.tensor.matmul(
                                sc[:, j * QBLK : (j + 1) * QBLK],
                                kT_sb[:, sk * P : (sk + 1) * P],
                                qT_sb[:, q_sl],
                                start=True,
                                stop=True,
                            )
                        ei = exp_pool.tile([P, GW], i16, tag="e")
                        ef = ei[:].bitcast(f16)
                        if g == OFF_GROUP:
                            # Schraudolph: fp16 exp bits via one tensor_scalar
                            nc.vector.tensor_scalar(
                                ei[:], sc[:], EXP_A * SCALE, EXP_B,
                                mult, add,
                            )
                        else:
                            nc.scalar.activation(
                                ef, sc[:],
                                mybir.ActivationFunctionType.Exp,
                                scale=SCALE,
                            )
                        e_tiles[g] = ef
                        if g == 1:
                            # normalize+store for q-block i-2 (its 1/sums
                            # finished on GPSIMD during q-block i-1)
                            flush_mul()
                        if g == 3:
                            # previous q-block's sums chain: emitted here so
                            # its acc dependency (complete shortly past the
                            # boundary) never stalls the PE stream
                            flush_sums()
                        # softmax-sum adds strictly in group order (the
                        # OFF_GROUP add follows its DVE exp immediately), so
                        # acc completes one add after the last ACT exp
                        if g == 1:
                            nc.vector.tensor_add(
                                acc[:], e_tiles[0], e_tiles[1]
                            )
                        elif g > 1:
                            nc.vector.tensor_add(acc[:], acc[:], ef)
                        pv_queue.append(
                            (g, ef, out_ps, v_sb,
                             g == 0, g == N_GROUPS - 1)
                        )
                        emit_pv_lagged()
                    pending_sums.append((acc, out_ps, pair, q_sl))

            emit_pv_lagged(drain=True)
            flush_sums()
            flush_mul()

    _split_excess_waits(nc)
    return nc


def _get_compiled():
    global _COMPILED
    if _COMPILED is None:
        _COMPILED = _build()
    return _COMPILED


def _round_f32r(x):
    """Round fp32 to the fp32r grid: round-to-nearest-even at 11 mantissa
    bits (verified bit-exact against the on-chip DVE fp32->fp32r cast)."""
    b = np.ascontiguousarray(x).view(np.uint32).astype(np.uint64)
    drop = np.uint64(12)
    half = np.uint64(1 << 11)
    lsb = (b >> drop) & np.uint64(1)
    r = (b + half - np.uint64(1) + lsb) & np.uint64(0xFFFFF000)
    return r.astype(np.uint32).view(np.float32).reshape(x.shape)


def _shard_inputs(query, key, value):
    """Full [B,S,H,D] inputs -> per-core input maps (host-side Ulysses)."""
    # [B,S,H,D] -> [B,H,D,S] -> [BH, D, S] for q/k; [B,H,S,D] -> [BH, S, D] for v
    qT_all = np.ascontiguousarray(np.transpose(query, (0, 2, 3, 1))).reshape(
        B * H, D, S
    )
    kT_all = np.ascontiguousarray(np.transpose(key, (0, 2, 3, 1))).reshape(
        B * H, D, S
    )
    v_all = np.ascontiguousarray(np.transpose(value, (0, 2, 1, 3))).reshape(
        B * H, S, D
    )
    in_maps = []
    for c in range(N_CORES):
        sl = slice(c * PAIRS_PER_CORE, (c + 1) * PAIRS_PER_CORE)
        in_maps.append(
            {
                "qT": np.ascontiguousarray(qT_all[sl]).astype(np.float16),
                "kT": np.ascontiguousarray(kT_all[sl]).astype(np.float16),
                "v": np.ascontiguousarray(v_all[sl]).astype(np.float16),
            }
        )
    return in_maps


def _gather_output(results):
    outT_all = np.concatenate([r["outT"] for r in results], axis=0)  # [BH, D, S]
    out = outT_all.reshape(B, H, D, S).transpose(0, 3, 1, 2)  # [B, S, H, D]
    return np.ascontiguousarray(out)


def kernel(query, key, value, _run_kwargs=None):
    from concourse.bass_utils import run_bass_kernel_spmd

    nc = _get_compiled()
    in_maps = _shard_inputs(
        np.asarray(query, dtype=np.float32),
        np.asarray(key, dtype=np.float32),
        np.asarray(value, dtype=np.float32),
    )
    kwargs = _run_kwargs or {}
    res = run_bass_kernel_spmd(nc, in_maps, core_ids=list(range(N_CORES)), **kwargs)
    out = _gather_output(res.results)
    if _run_kwargs is not None:
        kernel.last_result = res
    return out


# revision 17
# speedup vs baseline: 1.2369x; 1.0210x over previous
"""Multi-head attention (B=2, S=2048, H=16, D=128, fp32, non-causal) on 8
Trainium2 NeuronCores.

Strategy: the 32 (batch, head) pairs are independent -> head-parallel
(Ulysses-style) sharding, 4 pairs per core, no on-device collectives.
The host pre-transposes Q and K to [d, s] layout per pair (so the
contraction dim d lands on SBUF partitions with no on-chip transposes),
and the kernel produces out^T [d, s] which the host transposes back.

v2 engine balance (the v1 kernel was ACT-bound at 93% busy):
- exp of scores is split between the ACT engine (table exp, fp16 out) and
  the DVE via a Schraudolph bit-trick: one tensor_scalar computing
  round(score * SCALE*1024/ln2 + 15316) into an int16 tile whose bits ARE
  the fp16 exp approximation (max rel err ~3%, which perturbs the softmax
  output by <1e-3 of its max; conversion rounding verified on HW).
- softmax reciprocal moved off ACT entirely: 1/sums via a fast-inverse
  int32 bit-trick seed + one Newton step, all on the otherwise-idle GPSIMD
  engine (stock tensor ops; ~1.2e-3 rel err).
- the final normalize multiply is emitted one q-block late so the
  in-order DVE/GPSIMD queues never stall the steady-state exp pipeline.
- ACT runs a pure exp stream (no Ln/Exp reciprocal chain, no stalls).
"""

import math

import numpy as np

B, S, H, D = 2, 2048, 16, 128
N_CORES = 8
PAIRS_PER_CORE = (B * H) // N_CORES  # 4
P = 128
QBLK = 512  # q columns per q-block (one PSUM bank of fp32)
N_QB = S // QBLK  # 4
N_SK = S // P  # 16 sk tiles per pair
SK_PER_GROUP = 2  # sk tiles per scores/exp group ([128, 1024] psum tiles)
N_GROUPS = N_SK // SK_PER_GROUP  # 8
GW = SK_PER_GROUP * QBLK  # group width: 1024
SCALE = 1.0 / math.sqrt(D)

# group whose exp runs on DVE instead of ACT (None = all ACT). Its PV
# matmuls and softmax-sum add are deferred to the end of the q-block, so the
# DVE latency hides behind the other 7 groups' compute. Group 3 is chosen so
# the DVE op sits mid-stream, clear of the q-block-boundary critical path.
OFF_GROUP = -1

# Schraudolph fp16 exp: bits16 = round(x * EXP_A + EXP_B); bitcast -> fp16
EXP_A = 1024.0 / math.log(2.0)
EXP_B = 15360.0 - 44.0
# fast inverse seed: y0_bits = RECIP_MAGIC - bits(x)
RECIP_MAGIC = 0x7EF311C3

_COMPILED = None


def _patch_tile_drain():
    """Workaround for walrus 'Too many sync wait commands' on the TileContext
    tail Drain: redistribute all but one of the drain's sem waits onto
    single-wait NoOps on the sync engine (program order places them after the
    drain and before the all-engine barrier, which preserves semantics)."""
    import concourse.mybir as mybir
    import concourse.tile as tile
    from concourse.vector_clock import ScopedClock

    if getattr(tile.TileContext, "_ant_drain_patched", False):
        return

    def _drain_and_barrier(self, tick_clock, wait_clock):
        drain_inst = self.nc.sync.drain()
        wait_clock.add_sem_waits(
            drain_inst.ins, ScopedClock({None: tick_clock.global_clock})
        )
        si = drain_inst.ins.sync_info
        if si is not None and si.on_wait and len(si.on_wait) > 1:
            waits = list(si.on_wait)
            si.on_wait = waits[:1]
            # distribute the remaining waits round-robin across engines so
            # they are honored in parallel; the all-engine barrier below
            # collects them all before the semaphore reset
            engines = [
                self.nc.sync, self.nc.vector, self.nc.scalar,
                self.nc.tensor, self.nc.gpsimd,
            ]
            for i, w in enumerate(waits[1:]):
                nop = engines[i % len(engines)].nop(nofuse=True)
                nop.ins.sync_info = mybir.SyncInfo(on_wait=[w], on_update=[])

        self.nc.all_engine_barrier()
        assert self.sems is not None
        popped = self.nc._tile_sem_poison_stack.pop()
        assert popped is self._sem_poison
        self.nc.clear_and_free_semaphores(list(self.sems.allocated().values()))
        self.nc.all_engine_barrier()

    tile.TileContext._drain_and_barrier = _drain_and_barrier
    tile.TileContext._ant_drain_patched = True


def _split_excess_waits(nc):
    """This container's walrus rejects instructions carrying more than a
    struct-dependent number of semaphore waits (setupSyncWait: 'Too many
    sync wait commands'): 1 for Matmult/Ldweights (S3_LW struct), 2 for
    everything else. Hoist the excess onto NoOps inserted just before the
    instruction on the same engine — same-engine program order guarantees
    they are honored before the instruction issues."""
    import concourse.mybir as mybir

    seq = 0
    for f in nc.m.functions:
        for b in f.blocks:
            insts = list(b.instructions)
            out = []
            changed = False
            for inst in insts:
                max_waits = 1
                si = inst.sync_info
                if si is not None and si.on_wait and len(si.on_wait) > max_waits:
                    waits = list(si.on_wait)
                    si.on_wait = waits[:max_waits]
                    # NoOps (CTRL struct) only take 1 wait each
                    for w in waits[max_waits:]:
                        nop = mybir.InstNoOp(name=f"ant-waitsplit-{seq}")
                        seq += 1
                        nop.engine = inst.engine
                        nop.sync_info = mybir.SyncInfo(
                            on_wait=[w], on_update=[]
                        )
                        out.append(nop)
                    changed = True
                out.append(inst)
            if changed:
                b.instructions = out
    return nc


def _build():
    import concourse.bass as bass
    import concourse.mybir as mybir
    import concourse.tile as tile

    _patch_tile_drain()

    f32 = mybir.dt.float32
    f32r = mybir.dt.float32r
    f16 = mybir.dt.float16
    i16 = mybir.dt.int16
    i32 = mybir.dt.int32
    mult = mybir.AluOpType.mult
    add = mybir.AluOpType.add
    subtract = mybir.AluOpType.subtract
    nc = bass.Bass()

    # Q/K/V all arrive pre-cast to fp16 so every matmul runs in one uniform
    # PE mode (alternating fp32r/fp16 stationary modes measurably stalls the
    # PE pipeline), with half the DMA traffic. Scores quantization error from
    # fp16 Q/K is ~3e-4 -- far below the exp-approximation error budget.
    qT = nc.dram_tensor("qT", [PAIRS_PER_CORE, P, S], f16, kind="ExternalInput")
    kT = nc.dram_tensor("kT", [PAIRS_PER_CORE, P, S], f16, kind="ExternalInput")
    v = nc.dram_tensor("v", [PAIRS_PER_CORE, S, D], f16, kind="ExternalInput")
    outT = nc.dram_tensor("outT", [PAIRS_PER_CORE, P, S], f32, kind="ExternalOutput")

    with tile.TileContext(nc) as tc:
        with (
            tc.tile_pool(name="const", bufs=1) as const_pool,
            tc.tile_pool(name="inp", bufs=2) as inp_pool,
            tc.tile_pool(name="exp", bufs=8) as exp_pool,
            tc.tile_pool(name="acc", bufs=2) as acc_pool,
            tc.tile_pool(name="norm", bufs=2) as norm_pool,
            tc.tile_pool(name="outsb", bufs=3) as out_pool,
            tc.tile_pool(name="sc_ps", bufs=3, space="PSUM") as sc_psum,
            tc.tile_pool(name="o_ps", bufs=2, space="PSUM") as o_psum,
        ):
            ones_ld = const_pool.tile([P, P], f32)
            nc.vector.memset(ones_ld[:], 1.0)
            ones = const_pool.tile([P, P], f16)
            nc.vector.tensor_copy(ones[:], ones_ld[:])

            def emit_loads(pair):
                # chunked so the first scores matmuls start sooner: the
                # first q-block needs qT[:, :512] and kT tiles in order
                qT_sb = inp_pool.tile([P, S], f16, tag="qT")
                kT_sb = inp_pool.tile([P, S], f16, tag="kT")
                v_sb = inp_pool.tile([P, N_SK, D], f16, tag="v")
                nQ = 4
                for h in range(nQ):
                    sl = slice(h * (S // nQ), (h + 1) * (S // nQ))
                    nc.sync.dma_start(kT_sb[:, sl], kT[pair][:, sl])
                    if h == 0:
                        nc.sync.dma_start(qT_sb[:, sl], qT[pair][:, sl])
                rest = slice(S // nQ, S)
                nc.sync.dma_start(qT_sb[:, rest], qT[pair][:, rest])
                nc.sync.dma_start(
                    v_sb[:], v[pair].rearrange("(t p) d -> p t d", p=P)
                )
                return qT_sb, kT_sb, v_sb

            # pending normalize from the previous q-block:
            # (out_ps tile, -1/sums tile, pair idx, q slice)
            # deferred-by-one-q-block pipelines: nothing dependency-heavy is
            # ever emitted at the tail of a q-block, so no engine's in-order
            # queue blocks the next q-block's scores/exp stream.
            pending_sums = []  # (acc, out_ps, pair, q_sl) awaiting sums chain
            pending_mul = []   # (out_ps, y1n, pair, q_sl) awaiting normalize

            def flush_mul():
                while pending_mul:
                    out_ps, y1n, ppair, psl = pending_mul.pop(0)
                    o_sb = out_pool.tile([P, QBLK], f32, tag="osb")
                    nc.vector.tensor_mul(o_sb[:], out_ps[:], y1n[:])
                    nc.sync.dma_start(outT[ppair][:, psl], o_sb[:])

            def flush_sums():
                while pending_sums:
                    acc, out_ps, ppair, psl = pending_sums.pop(0)
                    # partition-reduce both acc halves into one PSUM bank
                    # (2-matmul accumulation; no separate fold op needed);
                    # the sums tile squats in the sc ring, keeping PSUM at
                    # exactly 8 banks with triple-buffered scores
                    sums_ps = sc_psum.tile([P, GW], f32, tag="sc")
                    nc.tensor.matmul(
                        sums_ps[:, :QBLK], ones[:], acc[:, :QBLK],
                        start=True, stop=False,
                    )
                    nc.tensor.matmul(
                        sums_ps[:, :QBLK], ones[:], acc[:, QBLK:],
                        start=False, stop=True,
                    )
                    sums_sb = norm_pool.tile([P, QBLK], f32, tag="sums_sb")
                    nc.vector.tensor_copy(sums_sb[:], sums_ps[:, :QBLK])
                    # fast inverse: bit-trick seed + 1 Newton step on GPSIMD
                    y0i = norm_pool.tile([P, QBLK], i32, tag="y0")
                    nc.gpsimd.tensor_scalar(
                        y0i[:], sums_sb[:].bitcast(i32), -1, RECIP_MAGIC,
                        mult, add,
                    )
                    y0f = y0i[:].bitcast(f32)
                    t2 = norm_pool.tile([P, QBLK], f32, tag="t2")
                    nc.gpsimd.tensor_tensor(t2[:], sums_sb[:], y0f, mult)
                    u2 = norm_pool.tile([P, QBLK], f32, tag="u2")
                    nc.gpsimd.tensor_scalar(
                        u2[:], t2[:], -1.0, 2.0, mult, add
                    )
                    y1n = norm_pool.tile([P, QBLK], f32, tag="y1n")
                    nc.gpsimd.tensor_tensor(y1n[:], u2[:], y0f, mult)
                    pending_mul.append((out_ps, y1n, ppair, psl))

            # One flat software pipeline over all (pair, qb, g) groups with a
            # constant 2-group PV lag: a group's PV matmuls are emitted two
            # group-slots after its scores, so the ~1.1us exp latency is
            # always covered and q-block boundaries never bubble (the next
            # q-block's scores are emitted before this one's last PVs).
            pv_queue = []  # (g, ef, out_ps, v_sb, start, stop)

            def emit_pv_lagged(drain=False):
                while len(pv_queue) > (0 if drain else 4):
                    g, ef, ops, vsb, st, sp = pv_queue.pop(0)
                    for j in range(SK_PER_GROUP):
                        sk = g * SK_PER_GROUP + j
                        nc.tensor.matmul(
                            ops[:],
                            vsb[:, sk, :],
                            ef[:, j * QBLK : (j + 1) * QBLK],
                            start=(st and j == 0),
                            stop=(sp and j == SK_PER_GROUP - 1),
                        )

            # software prefetch: emit the next pair's load DMAs before the
            # current pair's compute so transfers fully overlap it
            cur_tiles = emit_loads(0)
            for pair in range(PAIRS_PER_CORE):
                qT_sb, kT_sb, v_sb = cur_tiles
                if pair + 1 < PAIRS_PER_CORE:
                    cur_tiles = emit_loads(pair + 1)

                for qb in range(N_QB):
                    q_sl = slice(qb * QBLK, (qb + 1) * QBLK)
                    out_ps = o_psum.tile([P, QBLK], f32, tag="ops")
                    acc = acc_pool.tile([P, GW], f16, tag="acc")

                    e_tiles = [None] * N_GROUPS
                    for g in range(N_GROUPS):
                        sc = sc_psum.tile([P, GW], f32, tag="sc")
                        for j in range(SK_PER_GROUP):
                            sk = g * SK_PER_GROUP + j
                            nc.tensor.matmul(
                                sc[:, j * QBLK : (j + 1) * QBLK],
                                kT_sb[:, sk * P : (sk + 1) * P],
                                qT_sb[:, q_sl],
                                start=True,
                                stop=True,
                            )
                        ei = exp_pool.tile([P, GW], i16, tag="e")
                        ef = ei[:].bitcast(f16)
                        if g == OFF_GROUP:
                            # Schraudolph: fp16 exp bits via one tensor_scalar
                            nc.vector.tensor_scalar(
                                ei[:], sc[:], EXP_A * SCALE, EXP_B,
                                mult, add,
                            )
                        else:
                            nc.scalar.activation(
                                ef, sc[:],
                                mybir.ActivationFunctionType.Exp,
                                scale=SCALE,
                            )
                        e_tiles[g] = ef
                        if g == 1:
                            # normalize+store for q-block i-2 (its 1/sums
                            # finished on GPSIMD during q-block i-1)
                            flush_mul()
                        if g == 3:
                            # previous q-block's sums chain: emitted here so
                            # its acc dependency (complete shortly past the
                            # boundary) never stalls the PE stream
                            flush_sums()
                        # softmax-sum adds strictly in group order (the
                        # OFF_GROUP add follows its DVE exp immediately), so
                        # acc completes one add after the last ACT exp
                        if g == 1:
                            nc.vector.tensor_add(
                                acc[:], e_tiles[0], e_tiles[1]
                            )
                        elif g > 1:
                            nc.vector.tensor_add(acc[:], acc[:], ef)
                        pv_queue.append(
                            (g, ef, out_ps, v_sb,
                             g == 0, g == N_GROUPS - 1)
                        )
                        emit_pv_lagged()
                    pending_sums.append((acc, out_ps, pair, q_sl))

            emit_pv_lagged(drain=True)
            flush_sums()
            flush_mul()

    _split_excess_waits(nc)
    return nc


def _get_compiled():
    global _COMPILED
    if _COMPILED is None:
        _COMPILED = _build()
    return _COMPILED


def _round_f32r(x):
    """Round fp32 to the fp32r grid: round-to-nearest-even at 11 mantissa
    bits (verified bit-exact against the on-chip DVE fp32->fp32r cast)."""
    b = np.ascontiguousarray(x).view(np.uint32).astype(np.uint64)
    drop = np.uint64(12)
    half = np.uint64(1 << 11)
    lsb = (b >> drop) & np.uint64(1)
    r = (b + half - np.uint64(1) + lsb) & np.uint64(0xFFFFF000)
    return r.astype(np.uint32).view(np.float32).reshape(x.shape)


def _shard_inputs(query, key, value):
    """Full [B,S,H,D] inputs -> per-core input maps (host-side Ulysses)."""
    # [B,S,H,D] -> [B,H,D,S] -> [BH, D, S] for q/k; [B,H,S,D] -> [BH, S, D] for v
    qT_all = np.ascontiguousarray(np.transpose(query, (0, 2, 3, 1))).reshape(
        B * H, D, S
    )
    kT_all = np.ascontiguousarray(np.transpose(key, (0, 2, 3, 1))).reshape(
        B * H, D, S
    )
    v_all = np.ascontiguousarray(np.transpose(value, (0, 2, 1, 3))).reshape(
        B * H, S, D
    )
    in_maps = []
    for c in range(N_CORES):
        sl = slice(c * PAIRS_PER_CORE, (c + 1) * PAIRS_PER_CORE)
        in_maps.append(
            {
                "qT": np.ascontiguousarray(qT_all[sl]).astype(np.float16),
                "kT": np.ascontiguousarray(kT_all[sl]).astype(np.float16),
                "v": np.ascontiguousarray(v_all[sl]).astype(np.float16),
            }
        )
    return in_maps


def _gather_output(results):
    outT_all = np.concatenate([r["outT"] for r in results], axis=0)  # [BH, D, S]
    out = outT_all.reshape(B, H, D, S).transpose(0, 3, 1, 2)  # [B, S, H, D]
    return np.ascontiguousarray(out)


def kernel(query, key, value, _run_kwargs=None):
    from concourse.bass_utils import run_bass_kernel_spmd

    nc = _get_compiled()
    in_maps = _shard_inputs(
        np.asarray(query, dtype=np.float32),
        np.asarray(key, dtype=np.float32),
        np.asarray(value, dtype=np.float32),
    )
    kwargs = _run_kwargs or {}
    res = run_bass_kernel_spmd(nc, in_maps, core_ids=list(range(N_CORES)), **kwargs)
    out = _gather_output(res.results)
    if _run_kwargs is not None:
        kernel.last_result = res
    return out


# revision 18
# speedup vs baseline: 1.2864x; 1.0400x over previous
"""Multi-head attention (B=2, S=2048, H=16, D=128, fp32, non-causal) on 8
Trainium2 NeuronCores.

Strategy: the 32 (batch, head) pairs are independent -> head-parallel
(Ulysses-style) sharding, 4 pairs per core, no on-device collectives.
The host pre-transposes Q and K to [d, s] layout per pair (so the
contraction dim d lands on SBUF partitions with no on-chip transposes),
and the kernel produces out^T [d, s] which the host transposes back.

v2 engine balance (the v1 kernel was ACT-bound at 93% busy):
- exp of scores is split between the ACT engine (table exp, fp16 out) and
  the DVE via a Schraudolph bit-trick: one tensor_scalar computing
  round(score * SCALE*1024/ln2 + 15316) into an int16 tile whose bits ARE
  the fp16 exp approximation (max rel err ~3%, which perturbs the softmax
  output by <1e-3 of its max; conversion rounding verified on HW).
- softmax reciprocal moved off ACT entirely: 1/sums via a fast-inverse
  int32 bit-trick seed + one Newton step, all on the otherwise-idle GPSIMD
  engine (stock tensor ops; ~1.2e-3 rel err).
- the final normalize multiply is emitted one q-block late so the
  in-order DVE/GPSIMD queues never stall the steady-state exp pipeline.
- ACT runs a pure exp stream (no Ln/Exp reciprocal chain, no stalls).
"""

import math

import numpy as np

B, S, H, D = 2, 2048, 16, 128
N_CORES = 8
PAIRS_PER_CORE = (B * H) // N_CORES  # 4
P = 128
QBLK = 512  # q columns per q-block (one PSUM bank of fp32)
N_QB = S // QBLK  # 4
N_SK = S // P  # 16 sk tiles per pair
SK_PER_GROUP = 2  # sk tiles per scores/exp group ([128, 1024] psum tiles)
N_GROUPS = N_SK // SK_PER_GROUP  # 8
GW = SK_PER_GROUP * QBLK  # group width: 1024
SCALE = 1.0 / math.sqrt(D)

# group whose exp runs on DVE instead of ACT (None = all ACT). Its PV
# matmuls and softmax-sum add are deferred to the end of the q-block, so the
# DVE latency hides behind the other 7 groups' compute. Group 3 is chosen so
# the DVE op sits mid-stream, clear of the q-block-boundary critical path.
OFF_GROUP = -1

# Schraudolph fp16 exp: bits16 = round(x * EXP_A + EXP_B); bitcast -> fp16
EXP_A = 1024.0 / math.log(2.0)
EXP_B = 15360.0 - 44.0
# fast inverse seed: y0_bits = RECIP_MAGIC - bits(x)
RECIP_MAGIC = 0x7EF311C3

_COMPILED = None


def _patch_tile_drain():
    """Workaround for walrus 'Too many sync wait commands' on the TileContext
    tail Drain: redistribute all but one of the drain's sem waits onto
    single-wait NoOps on the sync engine (program order places them after the
    drain and before the all-engine barrier, which preserves semantics)."""
    import concourse.mybir as mybir
    import concourse.tile as tile
    from concourse.vector_clock import ScopedClock

    if getattr(tile.TileContext, "_ant_drain_patched", False):
        return

    def _drain_and_barrier(self, tick_clock, wait_clock):
        drain_inst = self.nc.sync.drain()
        wait_clock.add_sem_waits(
            drain_inst.ins, ScopedClock({None: tick_clock.global_clock})
        )
        si = drain_inst.ins.sync_info
        if si is not None and si.on_wait and len(si.on_wait) > 1:
            waits = list(si.on_wait)
            si.on_wait = waits[:1]
            # distribute the remaining waits round-robin across engines so
            # they are honored in parallel; the all-engine barrier below
            # collects them all before the semaphore reset
            engines = [
                self.nc.sync, self.nc.vector, self.nc.scalar,
                self.nc.tensor, self.nc.gpsimd,
            ]
            for i, w in enumerate(waits[1:]):
                nop = engines[i % len(engines)].nop(nofuse=True)
                nop.ins.sync_info = mybir.SyncInfo(on_wait=[w], on_update=[])

        self.nc.all_engine_barrier()
        assert self.sems is not None
        popped = self.nc._tile_sem_poison_stack.pop()
        assert popped is self._sem_poison
        self.nc.clear_and_free_semaphores(list(self.sems.allocated().values()))
        self.nc.all_engine_barrier()

    tile.TileContext._drain_and_barrier = _drain_and_barrier
    tile.TileContext._ant_drain_patched = True


def _split_excess_waits(nc):
    """This container's walrus rejects instructions carrying more than a
    struct-dependent number of semaphore waits (setupSyncWait: 'Too many
    sync wait commands'): 1 for Matmult/Ldweights (S3_LW struct), 2 for
    everything else. Hoist the excess onto NoOps inserted just before the
    instruction on the same engine — same-engine program order guarantees
    they are honored before the instruction issues."""
    import concourse.mybir as mybir

    seq = 0
    for f in nc.m.functions:
        for b in f.blocks:
            insts = list(b.instructions)
            out = []
            changed = False
            for inst in insts:
                max_waits = 1
                si = inst.sync_info
                if si is not None and si.on_wait and len(si.on_wait) > max_waits:
                    waits = list(si.on_wait)
                    si.on_wait = waits[:max_waits]
                    # NoOps (CTRL struct) only take 1 wait each
                    for w in waits[max_waits:]:
                        nop = mybir.InstNoOp(name=f"ant-waitsplit-{seq}")
                        seq += 1
                        nop.engine = inst.engine
                        nop.sync_info = mybir.SyncInfo(
                            on_wait=[w], on_update=[]
                        )
                        out.append(nop)
                    changed = True
                out.append(inst)
            if changed:
                b.instructions = out
    return nc


def _build():
    import concourse.bass as bass
    import concourse.mybir as mybir
    import concourse.tile as tile

    _patch_tile_drain()

    f32 = mybir.dt.float32
    f32r = mybir.dt.float32r
    f16 = mybir.dt.float16
    i16 = mybir.dt.int16
    i32 = mybir.dt.int32
    mult = mybir.AluOpType.mult
    add = mybir.AluOpType.add
    subtract = mybir.AluOpType.subtract
    nc = bass.Bass()

    # Q/K/V all arrive pre-cast to fp16 so every matmul runs in one uniform
    # PE mode (alternating fp32r/fp16 stationary modes measurably stalls the
    # PE pipeline), with half the DMA traffic. Scores quantization error from
    # fp16 Q/K is ~3e-4 -- far below the exp-approximation error budget.
    qT = nc.dram_tensor("qT", [PAIRS_PER_CORE, P, S], f16, kind="ExternalInput")
    kT = nc.dram_tensor("kT", [PAIRS_PER_CORE, P, S], f16, kind="ExternalInput")
    v = nc.dram_tensor("v", [PAIRS_PER_CORE, S, D], f16, kind="ExternalInput")
    outT = nc.dram_tensor("outT", [PAIRS_PER_CORE, P, S], f32, kind="ExternalOutput")

    with tile.TileContext(nc) as tc:
        with (
            tc.tile_pool(name="const", bufs=1) as const_pool,
            tc.tile_pool(name="inp", bufs=2) as inp_pool,
            tc.tile_pool(name="exp", bufs=8) as exp_pool,
            tc.tile_pool(name="acc", bufs=2) as acc_pool,
            tc.tile_pool(name="norm", bufs=2) as norm_pool,
            tc.tile_pool(name="outsb", bufs=3) as out_pool,
            tc.tile_pool(name="sc_ps", bufs=3, space="PSUM") as sc_psum,
            tc.tile_pool(name="o_ps", bufs=2, space="PSUM") as o_psum,
        ):
            ones_ld = const_pool.tile([P, P], f32)
            nc.vector.memset(ones_ld[:], 1.0)
            ones = const_pool.tile([P, P], f16)
            nc.vector.tensor_copy(ones[:], ones_ld[:])

            def emit_loads(pair):
                # chunked so the first scores matmuls start sooner: the
                # first q-block needs qT[:, :512] and kT tiles in order
                qT_sb = inp_pool.tile([P, S], f16, tag="qT")
                kT_sb = inp_pool.tile([P, S], f16, tag="kT")
                v_sb = inp_pool.tile([P, N_SK, D], f16, tag="v")
                nc.sync.dma_start(kT_sb[:, 0:256], kT[pair][:, 0:256])
                nc.sync.dma_start(qT_sb[:, 0:512], qT[pair][:, 0:512])
                nc.sync.dma_start(kT_sb[:, 256:1024], kT[pair][:, 256:1024])
                nc.sync.dma_start(kT_sb[:, 1024:S], kT[pair][:, 1024:S])
                nc.sync.dma_start(qT_sb[:, 512:S], qT[pair][:, 512:S])
                nc.sync.dma_start(
                    v_sb[:], v[pair].rearrange("(t p) d -> p t d", p=P)
                )
                return qT_sb, kT_sb, v_sb

            # pending normalize from the previous q-block:
            # (out_ps tile, -1/sums tile, pair idx, q slice)
            # deferred-by-one-q-block pipelines: nothing dependency-heavy is
            # ever emitted at the tail of a q-block, so no engine's in-order
            # queue blocks the next q-block's scores/exp stream.
            pending_sums = []  # (acc, out_ps, pair, q_sl) awaiting sums chain
            pending_mul = []   # (out_ps, y1n, pair, q_sl) awaiting normalize

            def flush_mul():
                while pending_mul:
                    out_ps, y1n, ppair, psl = pending_mul.pop(0)
                    o_sb = out_pool.tile([P, QBLK], f32, tag="osb")
                    nc.vector.tensor_mul(o_sb[:], out_ps[:], y1n[:])
                    nc.sync.dma_start(outT[ppair][:, psl], o_sb[:])

            def flush_sums(last=False):
                while pending_sums:
                    acc, out_ps, ppair, psl = pending_sums.pop(0)
                    # partition-reduce both acc halves into one PSUM bank
                    # (2-matmul accumulation; no separate fold op needed);
                    # the sums tile squats in the sc ring, keeping PSUM at
                    # exactly 8 banks with triple-buffered scores
                    sums_ps = sc_psum.tile([P, GW], f32, tag="sc")
                    nc.tensor.matmul(
                        sums_ps[:, :QBLK], ones[:], acc[:, :QBLK],
                        start=True, stop=False,
                    )
                    nc.tensor.matmul(
                        sums_ps[:, :QBLK], ones[:], acc[:, QBLK:],
                        start=False, stop=True,
                    )
                    if last:
                        # tail path: ACT is idle after the final exp, so
                        # 1/sums = exp(-ln(sums)) there is ~4us faster than
                        # waiting out the serial GPSIMD Newton chain
                        lns = norm_pool.tile([P, QBLK], f32, tag="t2")
                        nc.scalar.activation(
                            lns[:], sums_ps[:, :QBLK],
                            mybir.ActivationFunctionType.Ln,
                        )
                        y1n = norm_pool.tile([P, QBLK], f32, tag="y1n")
                        nc.scalar.activation(
                            y1n[:], lns[:],
                            mybir.ActivationFunctionType.Exp, scale=-1.0,
                        )
                        pending_mul.append((out_ps, y1n, ppair, psl))
                        continue
                    sums_sb = norm_pool.tile([P, QBLK], f32, tag="sums_sb")
                    nc.vector.tensor_copy(sums_sb[:], sums_ps[:, :QBLK])
                    # fast inverse: bit-trick seed + 1 Newton step on GPSIMD
                    y0i = norm_pool.tile([P, QBLK], i32, tag="y0")
                    nc.gpsimd.tensor_scalar(
                        y0i[:], sums_sb[:].bitcast(i32), -1, RECIP_MAGIC,
                        mult, add,
                    )
                    y0f = y0i[:].bitcast(f32)
                    t2 = norm_pool.tile([P, QBLK], f32, tag="t2")
                    nc.gpsimd.tensor_tensor(t2[:], sums_sb[:], y0f, mult)
                    u2 = norm_pool.tile([P, QBLK], f32, tag="u2")
                    nc.gpsimd.tensor_scalar(
                        u2[:], t2[:], -1.0, 2.0, mult, add
                    )
                    y1n = norm_pool.tile([P, QBLK], f32, tag="y1n")
                    nc.gpsimd.tensor_tensor(y1n[:], u2[:], y0f, mult)
                    pending_mul.append((out_ps, y1n, ppair, psl))

            # One flat software pipeline over all (pair, qb, g) groups with a
            # constant 2-group PV lag: a group's PV matmuls are emitted two
            # group-slots after its scores, so the ~1.1us exp latency is
            # always covered and q-block boundaries never bubble (the next
            # q-block's scores are emitted before this one's last PVs).
            pv_queue = []  # (g, ef, out_ps, v_sb, start, stop)

            def emit_pv_lagged(drain=False):
                while len(pv_queue) > (0 if drain else 4):
                    g, ef, ops, vsb, st, sp = pv_queue.pop(0)
                    for j in range(SK_PER_GROUP):
                        sk = g * SK_PER_GROUP + j
                        nc.tensor.matmul(
                            ops[:],
                            vsb[:, sk, :],
                            ef[:, j * QBLK : (j + 1) * QBLK],
                            start=(st and j == 0),
                            stop=(sp and j == SK_PER_GROUP - 1),
                        )

            # software prefetch: emit the next pair's load DMAs before the
            # current pair's compute so transfers fully overlap it
            cur_tiles = emit_loads(0)
            for pair in range(PAIRS_PER_CORE):
                qT_sb, kT_sb, v_sb = cur_tiles
                if pair + 1 < PAIRS_PER_CORE:
                    cur_tiles = emit_loads(pair + 1)

                for qb in range(N_QB):
                    q_sl = slice(qb * QBLK, (qb + 1) * QBLK)
                    out_ps = o_psum.tile([P, QBLK], f32, tag="ops")
                    acc = acc_pool.tile([P, GW], f16, tag="acc")

                    e_tiles = [None] * N_GROUPS
                    for g in range(N_GROUPS):
                        sc = sc_psum.tile([P, GW], f32, tag="sc")
                        for j in range(SK_PER_GROUP):
                            sk = g * SK_PER_GROUP + j
                            nc.tensor.matmul(
                                sc[:, j * QBLK : (j + 1) * QBLK],
                                kT_sb[:, sk * P : (sk + 1) * P],
                                qT_sb[:, q_sl],
                                start=True,
                                stop=True,
                            )
                        ei = exp_pool.tile([P, GW], i16, tag="e")
                        ef = ei[:].bitcast(f16)
                        if g == OFF_GROUP:
                            # Schraudolph: fp16 exp bits via one tensor_scalar
                            nc.vector.tensor_scalar(
                                ei[:], sc[:], EXP_A * SCALE, EXP_B,
                                mult, add,
                            )
                        else:
                            nc.scalar.activation(
                                ef, sc[:],
                                mybir.ActivationFunctionType.Exp,
                                scale=SCALE,
                            )
                        e_tiles[g] = ef
                        if g == 1:
                            # normalize+store for q-block i-2 (its 1/sums
                            # finished on GPSIMD during q-block i-1)
                            flush_mul()
                        if g == 3:
                            # previous q-block's sums chain: emitted here so
                            # its acc dependency (complete shortly past the
                            # boundary) never stalls the PE stream
                            flush_sums()
                        # softmax-sum adds strictly in group order (the
                        # OFF_GROUP add follows its DVE exp immediately), so
                        # acc completes one add after the last ACT exp
                        if g == 1:
                            nc.vector.tensor_add(
                                acc[:], e_tiles[0], e_tiles[1]
                            )
                        elif g > 1:
                            nc.vector.tensor_add(acc[:], acc[:], ef)
                        pv_queue.append(
                            (g, ef, out_ps, v_sb,
                             g == 0, g == N_GROUPS - 1)
                        )
                        emit_pv_lagged()
                    pending_sums.append((acc, out_ps, pair, q_sl))

            emit_pv_lagged(drain=True)
            flush_sums(last=True)
            flush_mul()

    _split_excess_waits(nc)
    return nc


def _get_compiled():
    global _COMPILED
    if _COMPILED is None:
        _COMPILED = _build()
    return _COMPILED


def _round_f32r(x):
    """Round fp32 to the fp32r grid: round-to-nearest-even at 11 mantissa
    bits (verified bit-exact against the on-chip DVE fp32->fp32r cast)."""
    b = np.ascontiguousarray(x).view(np.uint32).astype(np.uint64)
    drop = np.uint64(12)
    half = np.uint64(1 << 11)
    lsb = (b >> drop) & np.uint64(1)
    r = (b + half - np.uint64(1) + lsb) & np.uint64(0xFFFFF000)
    return r.astype(np.uint32).view(np.float32).reshape(x.shape)


def _shard_inputs(query, key, value):
    """Full [B,S,H,D] inputs -> per-core input maps (host-side Ulysses)."""
    # [B,S,H,D] -> [B,H,D,S] -> [BH, D, S] for q/k; [B,H,S,D] -> [BH, S, D] for v
    qT_all = np.ascontiguousarray(np.transpose(query, (0, 2, 3, 1))).reshape(
        B * H, D, S
    )
    kT_all = np.ascontiguousarray(np.transpose(key, (0, 2, 3, 1))).reshape(
        B * H, D, S
    )
    v_all = np.ascontiguousarray(np.transpose(value, (0, 2, 1, 3))).reshape(
        B * H, S, D
    )
    in_maps = []
    for c in range(N_CORES):
        sl = slice(c * PAIRS_PER_CORE, (c + 1) * PAIRS_PER_CORE)
        in_maps.append(
            {
                "qT": np.ascontiguousarray(qT_all[sl]).astype(np.float16),
                "kT": np.ascontiguousarray(kT_all[sl]).astype(np.float16),
                "v": np.ascontiguousarray(v_all[sl]).astype(np.float16),
            }
        )
    return in_maps


def _gather_output(results):
    outT_all = np.concatenate([r["outT"] for r in results], axis=0)  # [BH, D, S]
    out = outT_all.reshape(B, H, D, S).transpose(0, 3, 1, 2)  # [B, S, H, D]
    return np.ascontiguousarray(out)


def kernel(query, key, value, _run_kwargs=None):
    from concourse.bass_utils import run_bass_kernel_spmd

    nc = _get_compiled()
    in_maps = _shard_inputs(
        np.asarray(query, dtype=np.float32),
        np.asarray(key, dtype=np.float32),
        np.asarray(value, dtype=np.float32),
    )
    kwargs = _run_kwargs or {}
    res = run_bass_kernel_spmd(nc, in_maps, core_ids=list(range(N_CORES)), **kwargs)
    out = _gather_output(res.results)
    if _run_kwargs is not None:
        kernel.last_result = res
    return out
